# revision 30
# baseline (speedup 1.0000x reference)
"""Trainium2 Bass kernel for nn_DCDLayer (ragged_sequence).

Math (see reference):
    mean_f[b]  = mean of x2 rows in segment b                    [B, C]
    ha         = relu(BN(mean_f @ W1a) )  ; out_mean = relu(ha @ W2a)
    hb         = relu(BN(mean_f @ W1b) )  ; out_w    = sigmoid(relu(hb @ W2b))
    out[j]     = x2[j] * (0.5*out_w[seg j] + 0.75) + out_mean[seg j]

Sharding: 8 cores, each owns 8 whole segments (32768 contiguous rows of x2).

The kernel is HBM-bound: it must read x2 (64 MiB/core) and write out
(64 MiB/core); the only tunable traffic is re-materializing x2 for the
combine after the globally-coupled BN stats.  Strategy: during the phase-A
read every x2 tile is downcast to bf16 (rel err ~1e-3, tolerance 2e-2).
NRES tiles stay resident in SBUF; the rest spill to a bf16 DRAM scratch
(half the bytes of an fp32 re-read) and are re-loaded in phase C.

Per-core flow:
  phase A: DMA fp32 tile -> ACT downcast to bf16 -> PE colsum (bf16 matmul
           vs ones into PSUM fp32, accumulated per segment) -> spill-write
           the non-resident bf16 tiles.
  AllGather means [8,512] -> [64,512]  (BatchNorm couples all segments)
  MLP feature-sharded 8-ways (256-wide slice of MID per core, sliced on the
  host into its in_map, weights pre-cast to bf16), BN stats per-feature so
  they stay local; partial second matmuls AllReduce'd ([1024,64], tiny).
  phase C: out = x_bf16 * scale_bc[seg] + bias_bc[seg]
           (DVE mul + Pool add per tile), resident tiles first, then the
           spilled tiles re-loaded from the bf16 scratch.
"""

import sys
import numpy as np

for _p in ("/opt/trn_rl_repo",):
    if _p not in sys.path:
        sys.path.insert(0, _p)

B = 64            # segments
SEG = 4096        # rows per segment
N = B * SEG
C = 512
MID = 2048
EPS = 1e-5

NCORES = 8
B_LOC = B // NCORES          # 8 segments per core
ROWS = N // NCORES           # 32768 rows per core
FSH = MID // NCORES          # 256 features of MID per core
TPB = 4                      # 128-row tiles per DMA block (1 MiB fp32 blocks)
BLK_PER_SEG = SEG // (128 * TPB)   # 8 blocks per segment
NBLK = ROWS // (128 * TPB)   # 64 blocks per core
NRES = 34                    # bf16 tiles kept resident in SBUF
NSPILL = NBLK - NRES         # bf16 tiles spilled to DRAM scratch

_CACHE = {}


def _emit(nc, tc, tile, mybir, make_identity, t, collectives=True):
    f32 = mybir.dt.float32
    f32r = mybir.dt.float32r
    bf16 = mybir.dt.bfloat16
    Alu = mybir.AluOpType
    Act = mybir.ActivationFunctionType
    X = mybir.AxisListType.X
    RG = [list(range(NCORES))]

    from contextlib import ExitStack
    ctx = ExitStack()
    consts = ctx.enter_context(tc.tile_pool(name="consts", bufs=1))
    wpool = ctx.enter_context(tc.tile_pool(name="wpool", bufs=1))
    mlp = ctx.enter_context(tc.tile_pool(name="mlp", bufs=1))
    small = ctx.enter_context(tc.tile_pool(name="small", bufs=2))
    xio = ctx.enter_context(tc.tile_pool(name="xio", bufs=3))
    xsp = ctx.enter_context(tc.tile_pool(name="xsp", bufs=3))
    resp = ctx.enter_context(tc.tile_pool(name="resp", bufs=NRES))
    bcp = ctx.enter_context(tc.tile_pool(name="bcp", bufs=B_LOC))
    psA = ctx.enter_context(tc.tile_pool(name="psA", bufs=1, space="PSUM"))
    psB = ctx.enter_context(tc.tile_pool(name="psB", bufs=3, space="PSUM"))
    psC = ctx.enter_context(tc.tile_pool(name="psC", bufs=4, space="PSUM"))
    dram = ctx.enter_context(tc.tile_pool(name="dram", bufs=1, space="DRAM"))

    # ---- constants
    ones_fr = consts.tile([128, 1], f32r)   # 1/SEG folds the mean into colsum
    nc.gpsimd.memset(ones_fr, 1.0 / SEG)
    eps_col = consts.tile([128, 1], f32)
    nc.gpsimd.memset(eps_col, EPS)
    zero_col = consts.tile([128, 1], f32)
    nc.gpsimd.memset(zero_col, 0.0)
    # preload act tables: Sigmoid first, then Sqrt, so the sqrt table (which
    # also serves Copy/Square used through phase A) is resident when the BN
    # std is computed -- no table load on the means->scale critical chain.
    warm = consts.tile([1, 1], f32)
    nc.scalar.activation(warm, zero_col[:1, :], Act.Sigmoid, bias=zero_col[:1, :])
    nc.scalar.activation(warm, zero_col[:1, :], Act.Sqrt, bias=eps_col[:1, :])

    # ---- weights (per-core feature slices, bf16 from host) -> SBUF
    def load_w(name, ap, p_tiles, fdim):
        out = []
        for k in range(p_tiles):
            w = wpool.tile([128, fdim], bf16, tag=f"{name}{k}", name=f"{name}{k}")
            nc.sync.dma_start(w, ap[k * 128:(k + 1) * 128, :])
            out.append(w)
        return out

    w1a_sb = load_w("w1a", t["w1a"], 4, FSH)   # [512,256] -> 4x[128,256]
    w1b_sb = load_w("w1b", t["w1b"], 4, FSH)
    w2a_sb = load_w("w2a", t["w2a"], 2, C)     # [256,512] -> 2x[128,512]
    w2b_sb = load_w("w2b", t["w2b"], 2, C)

    def load_gb(name, vec):   # dram [FSH] -> SBUF [128, FSH//128] (feature on partition)
        o = mlp.tile([128, FSH // 128], f32, tag=f"{name}T", name=f"{name}T")
        nc.sync.dma_start(o, vec.rearrange("(a b) -> b a", a=FSH // 128))
        return o

    gaT = load_gb("ga", t["g1a"])
    baT = load_gb("ba", t["b1a"])
    gbT = load_gb("gb", t["g1b"])
    bbT = load_gb("bb", t["b1b"])

    xv = t["x"].rearrange("(n p) c -> p n c", p=128)    # [128, 256, 512]
    ov = t["out"].rearrange("(n p) c -> p n c", p=128)

    # bf16 DRAM scratch for the NSPILL spilled tiles
    spill = dram.tile([128, NSPILL * TPB, C], bf16)

    # ---- phase A: stream fp32, downcast to bf16, PE colsum per segment
    # tiles 0..NRES-1 resident; tiles NRES..NBLK-1 spill (their writes trail
    # the last phase-A load, filling the DMA gap while the MLP runs).
    bftiles = {}
    agin = dram.tile([B_LOC, C], bf16)
    agout = dram.tile([B, C], bf16,
                      addr_space="Shared" if collectives else "Local")
    ps_seg = None
    spill_w = {}
    last_load = [None]
    for nb in range(NBLK):
        s, blk = divmod(nb, BLK_PER_SEG)
        xt = xio.tile([128, TPB, C], f32, tag="xio", name=f"xa{nb}")
        last_load[0] = nc.sync.dma_start(xt, xv[:, nb * TPB:(nb + 1) * TPB, :])
        if nb < NRES:
            xb = resp.tile([128, TPB, C], bf16, tag="res", name=f"xres{nb}")
        else:
            xb = xsp.tile([128, TPB, C], bf16, tag="xsp", name=f"xsp{nb}")
        nc.scalar.copy(xb, xt)          # ACT fp32 -> bf16
        bftiles[nb] = xb
        if blk == 0:
            ps_seg = psA.tile([1, C], f32, tag="psA", name=f"psA{s}")
        # colsum straight off the fp32 tile as f32r (1 cycle/row, and the
        # mean chain does not wait for the ACT downcast)
        for k in range(TPB):
            nc.tensor.matmul(ps_seg, lhsT=ones_fr, rhs=xt[:, k, :].bitcast(f32r),
                             start=(blk == 0 and k == 0),
                             stop=(blk == BLK_PER_SEG - 1 and k == TPB - 1))
        if nb >= NRES:
            j = nb - NRES
            spill_w[nb] = nc.sync.dma_start(
                spill[:, j * TPB:(j + 1) * TPB, :], xb)
        if blk == BLK_PER_SEG - 1:
            msr = mlp.tile([1, C], bf16, tag="msr", name=f"msr{s}")
            nc.scalar.copy(msr, ps_seg)   # psum f32 -> bf16 mean row
            nc.sync.dma_start(agin[s:s + 1, :], msr)
    # keep the last few spill writes behind the final load: they fill the
    # DMA gap while the means MLP chain runs
    for nb in range(NBLK - 4, NBLK):
        tile.add_dep_helper(spill_w[nb].ins, last_load[0].ins, sync=True,
                            reason="cluster trailing spill writes in MLP gap")

    # ---- AllGather means
    if collectives:
        nc.gpsimd.collective_compute(
            "AllGather", Alu.bypass, replica_groups=RG,
            ins=[agin.opt()], outs=[agout.opt()],
        )
    else:
        nc.sync.dma_start(agout[:B_LOC, :], agin)
    # meansT via DMA xbar transpose: [64, 512] -> [512, 64] as [128, 4, 64],
    # so partition p / slot j holds feature c = 4p + j (W1 host rows are
    # permuted to match).
    mT_full = mlp.tile([128, 4, B], bf16)
    nc.sync.dma_start_transpose(mT_full, agout)
    mT = [mT_full[:, k, :] for k in range(4)]

    # ---- MLP branch: h1T = W1slice.T @ meansT ; BN per feature ; relu
    def branch(bid, w1_sb, gT, bT):
        haT = []
        for ml in range(FSH // 128):           # 2 local feature tiles
            ph = psB.tile([128, B], f32, tag="ps", name=f"ph{bid}{ml}")
            for k in range(4):
                nc.tensor.matmul(
                    ph, lhsT=w1_sb[k][:, ml * 128:(ml + 1) * 128], rhs=mT[k],
                    start=(k == 0), stop=(k == 3),
                )
            h = mlp.tile([128, B], f32, tag=f"h{bid}{ml}", name=f"h{bid}{ml}")
            s1 = small.tile([128, 1], f32, tag="s1", name=f"s1{bid}{ml}")
            nc.scalar.activation(h, ph, Act.Copy, accum_out=s1)
            sq = small.tile([128, B], f32, tag="sq", name=f"sq{bid}{ml}")
            s2 = small.tile([128, 1], f32, tag="s2", name=f"s2{bid}{ml}")
            nc.scalar.activation(sq, h, Act.Square, bias=zero_col, accum_out=s2)
            mu = small.tile([128, 1], f32, tag="mu", name=f"mu{bid}{ml}")
            nc.scalar.mul(mu, s1, 1.0 / B)
            ex2 = small.tile([128, 1], f32, tag="ex2", name=f"ex2{bid}{ml}")
            nc.scalar.mul(ex2, s2, 1.0 / B)
            # negvar = mu^2 - E[h^2]; std = sqrt(-negvar + eps)
            nv = small.tile([128, 1], f32, tag="nv", name=f"nv{bid}{ml}")
            nc.vector.scalar_tensor_tensor(nv, mu, mu, ex2,
                                           op0=Alu.mult, op1=Alu.subtract)
            std = small.tile([128, 1], f32, tag="std", name=f"std{bid}{ml}")
            nc.scalar.activation(std, nv, Act.Sqrt, bias=eps_col, scale=-1.0)
            istd = small.tile([128, 1], f32, tag="istd", name=f"istd{bid}{ml}")
            nc.vector.reciprocal(istd, std)
            sc = small.tile([128, 1], f32, tag="sc", name=f"sc{bid}{ml}")
            nc.vector.tensor_mul(sc, gT[:, ml:ml + 1], istd)
            t1 = small.tile([128, 1], f32, tag="t1", name=f"t1{bid}{ml}")
            nc.vector.tensor_mul(t1, mu, sc)
            bi = small.tile([128, 1], f32, tag="bi", name=f"bi{bid}{ml}")
            nc.vector.tensor_sub(bi, bT[:, ml:ml + 1], t1)
            ha = mlp.tile([128, B], bf16, tag=f"ha{bid}{ml}", name=f"ha{bid}{ml}")
            nc.scalar.activation(ha, h, Act.Relu, bias=bi, scale=sc)
            haT.append(ha)
        return haT

    haTa = branch("a", w1a_sb, gaT, baT)
    haTb = branch("b", w1b_sb, gbT, bbT)

    # ---- partial second matmuls in [B, C] layout (no output transposes):
    # po[b, c] = sum_mid haT[mid, b] * W2[mid, c]; branch a -> partitions
    # 0..63, branch b -> 64..127 of one PSUM tile.
    po_ps = psB.tile([128, C], f32, tag="ps", name="po_ps")
    for bi_, (w2_sb, haT) in enumerate([(w2a_sb, haTa), (w2b_sb, haTb)]):
        dst = po_ps[bi_ * B:(bi_ + 1) * B, :]
        for ml in range(FSH // 128):
            nc.tensor.matmul(
                dst, lhsT=haT[ml], rhs=w2_sb[ml],
                start=(ml == 0), stop=(ml == FSH // 128 - 1),
            )
    arin_sb = mlp.tile([128, C], bf16)
    nc.scalar.copy(arin_sb, po_ps)
    # dummy sigmoid: forces the sigmoid act-table load into the AllReduce
    # round-trip window (ACT is idle there) instead of the post chain
    nc.scalar.activation(warm, arin_sb[:1, :1], Act.Sigmoid, bias=zero_col[:1])
    arin = dram.tile([128, C], bf16)
    arout = dram.tile([128, C], bf16,
                      addr_space="Shared" if collectives else "Local")
    nc.sync.dma_start(arin, arin_sb)
    if collectives:
        nc.gpsimd.collective_compute(
            "AllReduce", Alu.add, replica_groups=RG,
            ins=[arin.opt()], outs=[arout.opt()],
        )
    else:
        nc.sync.dma_start(arout[:, :], arin)

    # ---- post-AR: bias rows stay raw (relu folds into the bbc broadcast),
    # scale rows get relu+sigmoid (the 0.5*x+0.75 affine folds into sbc).
    post_sb = mlp.tile([128, C], bf16)
    nc.sync.dma_start(post_sb, arout)
    ob = mlp.tile([128, C], bf16, tag="post_ob", name="ob")
    nc.scalar.activation(ob[B:, :], post_sb[B:, :], Act.Relu,
                         bias=zero_col[B:])
    ob2 = mlp.tile([128, C], bf16, tag="post_ob2", name="ob2")
    nc.scalar.activation(ob2[B:, :], ob[B:, :], Act.Sigmoid, bias=zero_col[B:])

    # ---- per-core replicated one-hot selector [128, 8, 128] (bf16):
    # rows 0..63 select the bias row, rows 64..127 the scale row.
    sel_sb = mlp.tile([128, B_LOC, 128], bf16)
    selv = t["sel"].rearrange("(p s) q -> p s q", s=B_LOC)
    nc.sync.dma_start(sel_sb, selv)

    # ---- per-segment broadcast scale/bias tiles, hoisted ahead of the tile
    # loop so segment boundaries never stall the phase-C pipeline (bufs=4
    # gives 4 segments of lookahead).
    # Pool (idle in phase C) does the psum->bf16 copies so they never queue
    # behind ACT's per-tile upconverts; relu/affine fold into them.
    sbcs, bbcs = [], []
    for s in range(B_LOC):
        pbs = psC.tile([128, C], f32, tag="psbc", name=f"pbs{s}")
        nc.tensor.matmul(pbs, lhsT=sel_sb[B:, s, :], rhs=ob2[B:, :],
                         start=True, stop=True)
        sbc = bcp.tile([128, C], bf16, tag="sbc", name=f"sbc{s}")
        nc.gpsimd.tensor_scalar(sbc, pbs, 0.5, 0.75, op0=Alu.mult, op1=Alu.add)
        pbb = psC.tile([128, C], f32, tag="psbc", name=f"pbb{s}")
        nc.tensor.matmul(pbb, lhsT=sel_sb[:B, s, :], rhs=post_sb[:B, :],
                         start=True, stop=True)
        bbc = bcp.tile([128, C], bf16, tag="bbc", name=f"bbc{s}")
        nc.gpsimd.tensor_scalar_max(bbc, pbb, 0.0)
        sbcs.append(sbc)
        bbcs.append(bbc)

    # ---- phase C: out = x_bf16 * scale_bc + bias_bc
    # tile order 0..NBLK-1 = residents first, then the spilled tiles (whose
    # bf16 re-loads prefetch while residents are processed).
    # All-bf16 DVE math (2x mode) in place on the x tile, then one ACT copy
    # upconverts to the fp32 store tile: DVE ~2.3us, ACT ~1.9us per tile,
    # both inside the 2.9us per-tile DMA store time.
    for nb in range(NBLK):
        s, blk = divmod(nb, BLK_PER_SEG)
        sbc_b = sbcs[s][:, None, :].broadcast_to([128, TPB, C])
        bbc_b = bbcs[s][:, None, :].broadcast_to([128, TPB, C])
        if nb < NRES:
            xb = bftiles[nb]
        else:
            j = nb - NRES
            xb = xsp.tile([128, TPB, C], bf16, tag="xsp", name=f"xrl{nb}")
            nc.sync.dma_start(xb, spill[:, j * TPB:(j + 1) * TPB, :])
        nc.vector.tensor_mul(xb, xb, sbc_b)          # DVE bf16, in place
        nc.vector.tensor_add(xb, xb, bbc_b)          # DVE bf16, in place
        ot = xio.tile([128, TPB, C], f32, tag="xio", name=f"xo{nb}")
        nc.scalar.copy(ot, xb)                       # ACT bf16 -> f32
        nc.sync.dma_start(ov[:, nb * TPB:(nb + 1) * TPB, :], ot)

    ctx.close()


def _build(num_devices=NCORES, collectives=True):
    key = ("nc", num_devices, collectives)
    if key in _CACHE:
        return _CACHE[key]
    import concourse.bacc as bacc
    import concourse.tile as tile
    from concourse import mybir
    from concourse.masks import make_identity

    f32 = mybir.dt.float32
    bf16 = mybir.dt.bfloat16
    nc = bacc.Bacc("TRN2", target_bir_lowering=False, debug=False,
                   enable_asserts=False, num_devices=num_devices)
    t = {
        "x": nc.dram_tensor("x", [ROWS, C], f32, kind="ExternalInput").ap(),
        "w1a": nc.dram_tensor("w1a", [C, FSH], bf16, kind="ExternalInput").ap(),
        "w2a": nc.dram_tensor("w2a", [FSH, C], bf16, kind="ExternalInput").ap(),
        "w1b": nc.dram_tensor("w1b", [C, FSH], bf16, kind="ExternalInput").ap(),
        "w2b": nc.dram_tensor("w2b", [FSH, C], bf16, kind="ExternalInput").ap(),
        "g1a": nc.dram_tensor("g1a", [FSH], f32, kind="ExternalInput").ap(),
        "b1a": nc.dram_tensor("b1a", [FSH], f32, kind="ExternalInput").ap(),
        "g1b": nc.dram_tensor("g1b", [FSH], f32, kind="ExternalInput").ap(),
        "b1b": nc.dram_tensor("b1b", [FSH], f32, kind="ExternalInput").ap(),
        "sel": nc.dram_tensor("sel", [128 * B_LOC, 128], bf16, kind="ExternalInput").ap(),
        "out": nc.dram_tensor("out", [ROWS, C], f32, kind="ExternalOutput").ap(),
    }
    with tile.TileContext(nc) as tc:
        _emit(nc, tc, tile, mybir, make_identity, t, collectives=collectives)
    nc.compile()
    _CACHE[key] = nc
    return nc


def _make_in_maps(x2, W1a, g1a, b1a, W2a, W1b, g1b, b1b, W2b):
    import ml_dtypes
    bf = ml_dtypes.bfloat16
    # W1 rows permuted to match the dma-transposed means layout: SBUF row
    # j*128 + p holds original input feature c = 4p + j.
    perm = (4 * np.arange(128)[None, :] + np.arange(4)[:, None]).reshape(-1)
    in_maps = []
    for c in range(NCORES):
        f0, f1 = c * FSH, (c + 1) * FSH
        # sel[h*64 + r, s, :] = 1 iff r == c*B_LOC + s  (h=0 bias, h=1 scale)
        sel = np.zeros((2, B, B_LOC, 128), np.float32)
        sel[:, c * B_LOC + np.arange(B_LOC), np.arange(B_LOC), :] = 1.0
        sel = sel.reshape(128 * B_LOC, 128)
        in_maps.append({
            "x": np.ascontiguousarray(x2[c * ROWS:(c + 1) * ROWS]),
            "w1a": np.ascontiguousarray(W1a[perm][:, f0:f1]).astype(bf),
            "w2a": np.ascontiguousarray(W2a[f0:f1, :]).astype(bf),
            "w1b": np.ascontiguousarray(W1b[perm][:, f0:f1]).astype(bf),
            "w2b": np.ascontiguousarray(W2b[f0:f1, :]).astype(bf),
            "g1a": np.ascontiguousarray(g1a[f0:f1]),
            "b1a": np.ascontiguousarray(b1a[f0:f1]),
            "g1b": np.ascontiguousarray(g1b[f0:f1]),
            "b1b": np.ascontiguousarray(b1b[f0:f1]),
            "sel": sel.astype(bf),
        })
    return in_maps


def _numpy_fallback(x2, npoint, W1a, g1a, b1a, W2a, W1b, g1b, b1b, W2b):
    n = x2.shape[0]
    b = npoint.shape[0]
    cum = np.cumsum(npoint)
    seg = np.searchsorted(cum, np.arange(n), side="right")
    counts = npoint.astype(x2.dtype)
    sums = np.zeros((b, x2.shape[1]), x2.dtype)
    np.add.at(sums, seg, x2)
    mean_f = sums / counts[:, None]

    def bn(h, g, bb):
        m = h.mean(0)
        v = h.var(0)
        return (h - m) / np.sqrt(v + EPS) * g + bb

    ha = np.maximum(bn(mean_f @ W1a, g1a, b1a), 0)
    out_mean = np.maximum(ha @ W2a, 0)
    hb = np.maximum(bn(mean_f @ W1b, g1b, b1b), 0)
    zw = np.maximum(hb @ W2b, 0)
    out_w = 1.0 / (1.0 + np.exp(-zw))
    return out_w[seg] * x2 * 0.5 + x2 * 0.75 + out_mean[seg]


def run_on_device(inputs, trace=False, **kwargs):
    """Returns (full_output, BassKernelResults)."""
    from concourse import bass_utils
    x2 = np.asarray(inputs["x2"], np.float32)
    args = {k: np.asarray(inputs[k], np.float32)
            for k in ("W1a", "g1a", "b1a", "W2a", "W1b", "g1b", "b1b", "W2b")}
    nc = _build()
    in_maps = _make_in_maps(x2, args["W1a"], args["g1a"], args["b1a"],
                            args["W2a"], args["W1b"], args["g1b"],
                            args["b1b"], args["W2b"])
    res = bass_utils.run_bass_kernel_spmd(
        nc, in_maps, core_ids=list(range(NCORES)), trace=trace, **kwargs)
    out = np.concatenate([res.results[c]["out"] for c in range(NCORES)], axis=0)
    return out, res


def bench_device(inputs, iters=10, warmup=2, chain=1):
    """Time the sharded NEFF execution with inputs pre-staged on device.

    chain=N runs the kernel N times back-to-back inside one dispatch (each
    call's output feeds the next call's x), so per-call device time can be
    separated from the ~80ms axon dispatch floor via (T(N)-T(1))/(N-1).

    Returns (times_sec_list, output). Mirrors bass2jax.run_bass_via_pjrt's
    multi-core path but without donation so the callable can be re-invoked.
    """
    import time
    import jax
    from jax.experimental.shard_map import shard_map
    from jax.sharding import Mesh, NamedSharding, PartitionSpec
    from concourse import bass2jax, mybir

    nc = _build()
    x2 = np.asarray(inputs["x2"], np.float32)
    args = {k: np.asarray(inputs[k], np.float32)
            for k in ("W1a", "g1a", "b1a", "W2a", "W1b", "g1b", "b1b", "W2b")}
    in_maps = _make_in_maps(x2, args["W1a"], args["g1a"], args["b1a"],
                            args["W2a"], args["W1b"], args["g1b"],
                            args["b1b"], args["W2b"])

    bass2jax.install_neuronx_cc_hook()
    partition_name = (nc.partition_id_tensor.name
                      if nc.partition_id_tensor else None)
    in_names, out_names, out_avals, zero_outs = [], [], [], []
    for alloc in nc.m.functions[0].allocations:
        if not isinstance(alloc, mybir.MemoryLocationSet):
            continue
        name = alloc.memorylocations[0].name
        if alloc.kind == "ExternalInput":
            if name != partition_name:
                in_names.append(name)
        elif alloc.kind == "ExternalOutput":
            shape = tuple(alloc.tensor_shape)
            dtype = mybir.dt.np(alloc.dtype)
            out_names.append(name)
            out_avals.append(jax.core.ShapedArray(shape, dtype))
            zero_outs.append(np.zeros(shape, dtype))
    n_params = len(in_names)
    all_in_names = list(in_names) + list(out_names)
    if partition_name is not None:
        all_in_names.append(partition_name)

    xi = in_names.index("x")

    def _body(*a):
        operands = list(a)
        if partition_name is not None:
            operands.append(bass2jax.partition_id_tensor())
        for _ in range(chain):
            outs = bass2jax._bass_exec_p.bind(
                *operands,
                out_avals=tuple(out_avals),
                in_names=tuple(all_in_names),
                out_names=tuple(out_names),
                lowering_input_output_aliases=(),
                sim_require_finite=True,
                sim_require_nnan=True,
                nc=nc,
            )
            operands[xi] = outs[0]
        return tuple(outs)

    devices = jax.devices()[:NCORES]
    mesh = Mesh(np.asarray(devices), ("core",))
    spec = PartitionSpec("core")
    n_outs = len(out_names)
    fn = jax.jit(
        shard_map(_body, mesh=mesh,
                  in_specs=(spec,) * (n_params + n_outs),
                  out_specs=(spec,) * n_outs, check_rep=False),
        keep_unused=True,
    )
    sharding = NamedSharding(mesh, spec)
    concat_in = [
        jax.device_put(
            np.concatenate([np.asarray(in_maps[c][nm]) for c in range(NCORES)],
                           axis=0), sharding)
        for nm in in_names
    ]
    concat_zero = [
        jax.device_put(np.zeros((NCORES * z.shape[0], *z.shape[1:]), z.dtype),
                       sharding)
        for z in zero_outs
    ]
    for _ in range(warmup):
        r = fn(*concat_in, *concat_zero)
        jax.block_until_ready(r)
    times = []
    for _ in range(iters):
        t0 = time.perf_counter()
        r = fn(*concat_in, *concat_zero)
        jax.block_until_ready(r)
        times.append(time.perf_counter() - t0)
    out = np.asarray(r[0]).reshape(NCORES, ROWS, C).reshape(N, C)
    return times, out


def kernel(**inputs):
    x2 = np.asarray(inputs["x2"], np.float32)
    npoint = np.asarray(inputs["npoint"])
    if (x2.shape != (N, C) or npoint.shape != (B,)
            or not np.all(npoint == SEG)):
        return _numpy_fallback(
            x2, npoint,
            *[np.asarray(inputs[k], np.float32)
              for k in ("W1a", "g1a", "b1a", "W2a", "W1b", "g1b", "b1b", "W2b")],
        ).astype(np.float32)
    out, _ = run_on_device(inputs)
    return out


# revision 32
# speedup vs baseline: 1.0317x; 1.0317x over previous
"""Trainium2 Bass kernel for nn_DCDLayer (ragged_sequence).

Math (see reference):
    mean_f[b]  = mean of x2 rows in segment b                    [B, C]
    ha         = relu(BN(mean_f @ W1a) )  ; out_mean = relu(ha @ W2a)
    hb         = relu(BN(mean_f @ W1b) )  ; out_w    = sigmoid(relu(hb @ W2b))
    out[j]     = x2[j] * (0.5*out_w[seg j] + 0.75) + out_mean[seg j]

Sharding: 8 cores, each owns 8 whole segments (32768 contiguous rows of x2).

The kernel is HBM-bound: it must read x2 (64 MiB/core) and write out
(64 MiB/core); the only tunable traffic is re-materializing x2 for the
combine after the globally-coupled BN stats.  Strategy: during the phase-A
read every x2 tile is downcast to bf16 (rel err ~1e-3, tolerance 2e-2).
NRES tiles stay resident in SBUF; the rest spill to a bf16 DRAM scratch
(half the bytes of an fp32 re-read) and are re-loaded in phase C.

Per-core flow:
  phase A: DMA fp32 tile -> ACT downcast to bf16 -> PE colsum (bf16 matmul
           vs ones into PSUM fp32, accumulated per segment) -> spill-write
           the non-resident bf16 tiles.
  AllGather means [8,512] -> [64,512]  (BatchNorm couples all segments)
  MLP feature-sharded 8-ways (256-wide slice of MID per core, sliced on the
  host into its in_map, weights pre-cast to bf16), BN stats per-feature so
  they stay local; partial second matmuls AllReduce'd ([1024,64], tiny).
  phase C: out = x_bf16 * scale_bc[seg] + bias_bc[seg]
           (DVE mul + Pool add per tile), resident tiles first, then the
           spilled tiles re-loaded from the bf16 scratch.
"""

import sys
import numpy as np

for _p in ("/opt/trn_rl_repo",):
    if _p not in sys.path:
        sys.path.insert(0, _p)

B = 64            # segments
SEG = 4096        # rows per segment
N = B * SEG
C = 512
MID = 2048
EPS = 1e-5

NCORES = 8
B_LOC = B // NCORES          # 8 segments per core
ROWS = N // NCORES           # 32768 rows per core
FSH = MID // NCORES          # 256 features of MID per core
TPB = 4                      # 128-row tiles per DMA block (1 MiB fp32 blocks)
BLK_PER_SEG = SEG // (128 * TPB)   # 8 blocks per segment
NBLK = ROWS // (128 * TPB)   # 64 blocks per core
NRES = 33                    # bf16 tiles kept resident in SBUF
NSPILL = NBLK - NRES         # bf16 tiles spilled to DRAM scratch

_CACHE = {}


def _emit(nc, tc, tile, mybir, make_identity, t, collectives=True):
    f32 = mybir.dt.float32
    f32r = mybir.dt.float32r
    bf16 = mybir.dt.bfloat16
    Alu = mybir.AluOpType
    Act = mybir.ActivationFunctionType
    X = mybir.AxisListType.X
    RG = [list(range(NCORES))]

    from contextlib import ExitStack
    ctx = ExitStack()
    consts = ctx.enter_context(tc.tile_pool(name="consts", bufs=1))
    wpool = ctx.enter_context(tc.tile_pool(name="wpool", bufs=1))
    mlp = ctx.enter_context(tc.tile_pool(name="mlp", bufs=1))
    small = ctx.enter_context(tc.tile_pool(name="small", bufs=2))
    xio = ctx.enter_context(tc.tile_pool(name="xio", bufs=3))
    xsp = ctx.enter_context(tc.tile_pool(name="xsp", bufs=4))
    resp = ctx.enter_context(tc.tile_pool(name="resp", bufs=NRES))
    bcp = ctx.enter_context(tc.tile_pool(name="bcp", bufs=B_LOC))
    psA = ctx.enter_context(tc.tile_pool(name="psA", bufs=1, space="PSUM"))
    psB = ctx.enter_context(tc.tile_pool(name="psB", bufs=3, space="PSUM"))
    psC = ctx.enter_context(tc.tile_pool(name="psC", bufs=4, space="PSUM"))
    dram = ctx.enter_context(tc.tile_pool(name="dram", bufs=1, space="DRAM"))

    # ---- constants
    ones_fr = consts.tile([128, 1], f32r)   # 1/SEG folds the mean into colsum
    nc.gpsimd.memset(ones_fr, 1.0 / SEG)
    eps_col = consts.tile([128, 1], f32)
    nc.gpsimd.memset(eps_col, EPS)
    zero_col = consts.tile([128, 1], f32)
    nc.gpsimd.memset(zero_col, 0.0)
    # preload act tables: Sigmoid first, then Sqrt, so the sqrt table (which
    # also serves Copy/Square used through phase A) is resident when the BN
    # std is computed -- no table load on the means->scale critical chain.
    warm = consts.tile([1, 1], f32)
    nc.scalar.activation(warm, zero_col[:1, :], Act.Sigmoid, bias=zero_col[:1, :])
    nc.scalar.activation(warm, zero_col[:1, :], Act.Sqrt, bias=eps_col[:1, :])

    # ---- weights (per-core feature slices, bf16 from host) -> SBUF
    def load_w(name, ap, p_tiles, fdim):
        out = []
        for k in range(p_tiles):
            w = wpool.tile([128, fdim], bf16, tag=f"{name}{k}", name=f"{name}{k}")
            nc.sync.dma_start(w, ap[k * 128:(k + 1) * 128, :])
            out.append(w)
        return out

    w1a_sb = load_w("w1a", t["w1a"], 4, FSH)   # [512,256] -> 4x[128,256]
    w1b_sb = load_w("w1b", t["w1b"], 4, FSH)
    w2a_sb = load_w("w2a", t["w2a"], 2, C)     # [256,512] -> 2x[128,512]
    w2b_sb = load_w("w2b", t["w2b"], 2, C)

    def load_gb(name, vec):   # dram [FSH] -> SBUF [128, FSH//128] (feature on partition)
        o = mlp.tile([128, FSH // 128], f32, tag=f"{name}T", name=f"{name}T")
        nc.sync.dma_start(o, vec.rearrange("(a b) -> b a", a=FSH // 128))
        return o

    gaT = load_gb("ga", t["g1a"])
    baT = load_gb("ba", t["b1a"])
    gbT = load_gb("gb", t["g1b"])
    bbT = load_gb("bb", t["b1b"])

    xv = t["x"].rearrange("(n p) c -> p n c", p=128)    # [128, 256, 512]
    ov = t["out"].rearrange("(n p) c -> p n c", p=128)

    # bf16 DRAM scratch for the NSPILL spilled tiles
    spill = dram.tile([128, NSPILL * TPB, C], bf16)

    # ---- phase A: stream fp32, downcast to bf16, PE colsum per segment
    # tiles 0..NRES-1 resident; tiles NRES..NBLK-1 spill (their writes trail
    # the last phase-A load, filling the DMA gap while the MLP runs).
    bftiles = {}
    agin = dram.tile([B_LOC, C], bf16)
    agout = dram.tile([B, C], bf16,
                      addr_space="Shared" if collectives else "Local")
    ps_seg = None
    spill_w = {}
    last_load = [None]
    for nb in range(NBLK):
        s, blk = divmod(nb, BLK_PER_SEG)
        xt = xio.tile([128, TPB, C], f32, tag="xio", name=f"xa{nb}")
        last_load[0] = nc.sync.dma_start(xt, xv[:, nb * TPB:(nb + 1) * TPB, :])
        if nb < NRES:
            xb = resp.tile([128, TPB, C], bf16, tag="res", name=f"xres{nb}")
        else:
            xb = xsp.tile([128, TPB, C], bf16, tag="xsp", name=f"xsp{nb}")
        nc.scalar.copy(xb, xt)          # ACT fp32 -> bf16
        bftiles[nb] = xb
        if blk == 0:
            ps_seg = psA.tile([1, C], f32, tag="psA", name=f"psA{s}")
        # colsum straight off the fp32 tile as f32r (1 cycle/row, and the
        # mean chain does not wait for the ACT downcast)
        for k in range(TPB):
            nc.tensor.matmul(ps_seg, lhsT=ones_fr, rhs=xt[:, k, :].bitcast(f32r),
                             start=(blk == 0 and k == 0),
                             stop=(blk == BLK_PER_SEG - 1 and k == TPB - 1))
        if nb >= NRES:
            j = nb - NRES
            spill_w[nb] = nc.sync.dma_start(
                spill[:, j * TPB:(j + 1) * TPB, :], xb)
        if blk == BLK_PER_SEG - 1:
            msr = mlp.tile([1, C], bf16, tag="msr", name=f"msr{s}")
            nc.scalar.copy(msr, ps_seg)   # psum f32 -> bf16 mean row
            nc.sync.dma_start(agin[s:s + 1, :], msr)
    # keep the last few spill writes behind the final load: they fill the
    # DMA gap while the means MLP chain runs
    for nb in range(NBLK - 4, NBLK):
        tile.add_dep_helper(spill_w[nb].ins, last_load[0].ins, sync=True,
                            reason="cluster trailing spill writes in MLP gap")

    # ---- AllGather means
    if collectives:
        nc.gpsimd.collective_compute(
            "AllGather", Alu.bypass, replica_groups=RG,
            ins=[agin.opt()], outs=[agout.opt()],
        )
    else:
        nc.sync.dma_start(agout[:B_LOC, :], agin)
    # meansT via DMA xbar transpose: [64, 512] -> [512, 64] as [128, 4, 64],
    # so partition p / slot j holds feature c = 4p + j (W1 host rows are
    # permuted to match).
    mT_full = mlp.tile([128, 4, B], bf16)
    nc.sync.dma_start_transpose(mT_full, agout)
    mT = [mT_full[:, k, :] for k in range(4)]

    # ---- MLP branch: h1T = W1slice.T @ meansT ; BN per feature ; relu
    def branch(bid, w1_sb, gT, bT):
        haT = []
        for ml in range(FSH // 128):           # 2 local feature tiles
            ph = psB.tile([128, B], f32, tag="ps", name=f"ph{bid}{ml}")
            for k in range(4):
                nc.tensor.matmul(
                    ph, lhsT=w1_sb[k][:, ml * 128:(ml + 1) * 128], rhs=mT[k],
                    start=(k == 0), stop=(k == 3),
                )
            h = mlp.tile([128, B], f32, tag=f"h{bid}{ml}", name=f"h{bid}{ml}")
            s1 = small.tile([128, 1], f32, tag="s1", name=f"s1{bid}{ml}")
            nc.scalar.activation(h, ph, Act.Copy, accum_out=s1)
            sq = small.tile([128, B], f32, tag="sq", name=f"sq{bid}{ml}")
            s2 = small.tile([128, 1], f32, tag="s2", name=f"s2{bid}{ml}")
            nc.scalar.activation(sq, h, Act.Square, bias=zero_col, accum_out=s2)
            mu = small.tile([128, 1], f32, tag="mu", name=f"mu{bid}{ml}")
            nc.scalar.mul(mu, s1, 1.0 / B)
            ex2 = small.tile([128, 1], f32, tag="ex2", name=f"ex2{bid}{ml}")
            nc.scalar.mul(ex2, s2, 1.0 / B)
            # negvar = mu^2 - E[h^2]; std = sqrt(-negvar + eps)
            nv = small.tile([128, 1], f32, tag="nv", name=f"nv{bid}{ml}")
            nc.vector.scalar_tensor_tensor(nv, mu, mu, ex2,
                                           op0=Alu.mult, op1=Alu.subtract)
            std = small.tile([128, 1], f32, tag="std", name=f"std{bid}{ml}")
            nc.scalar.activation(std, nv, Act.Sqrt, bias=eps_col, scale=-1.0)
            istd = small.tile([128, 1], f32, tag="istd", name=f"istd{bid}{ml}")
            nc.vector.reciprocal(istd, std)
            sc = small.tile([128, 1], f32, tag="sc", name=f"sc{bid}{ml}")
            nc.vector.tensor_mul(sc, gT[:, ml:ml + 1], istd)
            t1 = small.tile([128, 1], f32, tag="t1", name=f"t1{bid}{ml}")
            nc.vector.tensor_mul(t1, mu, sc)
            bi = small.tile([128, 1], f32, tag="bi", name=f"bi{bid}{ml}")
            nc.vector.tensor_sub(bi, bT[:, ml:ml + 1], t1)
            ha = mlp.tile([128, B], bf16, tag=f"ha{bid}{ml}", name=f"ha{bid}{ml}")
            nc.scalar.activation(ha, h, Act.Relu, bias=bi, scale=sc)
            haT.append(ha)
        return haT

    haTa = branch("a", w1a_sb, gaT, baT)
    haTb = branch("b", w1b_sb, gbT, bbT)

    # ---- partial second matmuls in [B, C] layout (no output transposes):
    # po[b, c] = sum_mid haT[mid, b] * W2[mid, c]; branch a -> partitions
    # 0..63, branch b -> 64..127 of one PSUM tile.
    po_ps = psB.tile([128, C], f32, tag="ps", name="po_ps")
    for bi_, (w2_sb, haT) in enumerate([(w2a_sb, haTa), (w2b_sb, haTb)]):
        dst = po_ps[bi_ * B:(bi_ + 1) * B, :]
        for ml in range(FSH // 128):
            nc.tensor.matmul(
                dst, lhsT=haT[ml], rhs=w2_sb[ml],
                start=(ml == 0), stop=(ml == FSH // 128 - 1),
            )
    arin_sb = mlp.tile([128, C], bf16)
    nc.scalar.copy(arin_sb, po_ps)
    # dummy sigmoid: forces the sigmoid act-table load into the AllReduce
    # round-trip window (ACT is idle there) instead of the post chain
    nc.scalar.activation(warm, arin_sb[:1, :1], Act.Sigmoid, bias=zero_col[:1])
    arin = dram.tile([128, C], bf16)
    arout = dram.tile([128, C], bf16,
                      addr_space="Shared" if collectives else "Local")
    nc.sync.dma_start(arin, arin_sb)
    if collectives:
        nc.gpsimd.collective_compute(
            "AllReduce", Alu.add, replica_groups=RG,
            ins=[arin.opt()], outs=[arout.opt()],
        )
    else:
        nc.sync.dma_start(arout[:, :], arin)

    # ---- post-AR: bias rows stay raw (relu folds into the bbc broadcast),
    # scale rows get relu+sigmoid (the 0.5*x+0.75 affine folds into sbc).
    post_sb = mlp.tile([128, C], bf16)
    nc.sync.dma_start(post_sb, arout)
    ob = mlp.tile([128, C], bf16, tag="post_ob", name="ob")
    nc.scalar.activation(ob[B:, :], post_sb[B:, :], Act.Relu,
                         bias=zero_col[B:])
    ob2 = mlp.tile([128, C], bf16, tag="post_ob2", name="ob2")
    nc.scalar.activation(ob2[B:, :], ob[B:, :], Act.Sigmoid, bias=zero_col[B:])

    # ---- per-core replicated one-hot selector [128, 8, 128] (bf16):
    # rows 0..63 select the bias row, rows 64..127 the scale row.
    sel_sb = mlp.tile([128, B_LOC, 128], bf16)
    selv = t["sel"].rearrange("(p s) q -> p s q", s=B_LOC)
    nc.sync.dma_start(sel_sb, selv)

    # ---- per-segment broadcast scale/bias tiles, hoisted ahead of the tile
    # loop so segment boundaries never stall the phase-C pipeline (bufs=4
    # gives 4 segments of lookahead).
    # Pool (idle in phase C) does the psum->bf16 copies so they never queue
    # behind ACT's per-tile upconverts; relu/affine fold into them.
    sbcs, bbcs = [], []
    for s in range(B_LOC):
        pbs = psC.tile([128, C], f32, tag="psbc", name=f"pbs{s}")
        nc.tensor.matmul(pbs, lhsT=sel_sb[B:, s, :], rhs=ob2[B:, :],
                         start=True, stop=True)
        sbc = bcp.tile([128, C], bf16, tag="sbc", name=f"sbc{s}")
        nc.gpsimd.tensor_scalar(sbc, pbs, 0.5, 0.75, op0=Alu.mult, op1=Alu.add)
        pbb = psC.tile([128, C], f32, tag="psbc", name=f"pbb{s}")
        nc.tensor.matmul(pbb, lhsT=sel_sb[:B, s, :], rhs=post_sb[:B, :],
                         start=True, stop=True)
        bbc = bcp.tile([128, C], bf16, tag="bbc", name=f"bbc{s}")
        nc.gpsimd.tensor_scalar_max(bbc, pbb, 0.0)
        sbcs.append(sbc)
        bbcs.append(bbc)

    # ---- phase C: out = x_bf16 * scale_bc + bias_bc
    # tile order 0..NBLK-1 = residents first, then the spilled tiles (whose
    # bf16 re-loads prefetch while residents are processed).
    # All-bf16 DVE math (2x mode) in place on the x tile, then one ACT copy
    # upconverts to the fp32 store tile: DVE ~2.3us, ACT ~1.9us per tile,
    # both inside the 2.9us per-tile DMA store time.
    for nb in range(NBLK):
        s, blk = divmod(nb, BLK_PER_SEG)
        sbc_b = sbcs[s][:, None, :].broadcast_to([128, TPB, C])
        bbc_b = bbcs[s][:, None, :].broadcast_to([128, TPB, C])
        if nb < NRES:
            xb = bftiles[nb]
        else:
            j = nb - NRES
            xb = xsp.tile([128, TPB, C], bf16, tag="xsp", name=f"xrl{nb}")
            nc.sync.dma_start(xb, spill[:, j * TPB:(j + 1) * TPB, :])
        nc.vector.tensor_mul(xb, xb, sbc_b)          # DVE bf16, in place
        nc.vector.tensor_add(xb, xb, bbc_b)          # DVE bf16, in place
        ot = xio.tile([128, TPB, C], f32, tag="xio", name=f"xo{nb}")
        nc.scalar.copy(ot, xb)                       # ACT bf16 -> f32
        nc.sync.dma_start(ov[:, nb * TPB:(nb + 1) * TPB, :], ot)

    ctx.close()


def _build(num_devices=NCORES, collectives=True):
    key = ("nc", num_devices, collectives)
    if key in _CACHE:
        return _CACHE[key]
    import concourse.bacc as bacc
    import concourse.tile as tile
    from concourse import mybir
    from concourse.masks import make_identity

    f32 = mybir.dt.float32
    bf16 = mybir.dt.bfloat16
    nc = bacc.Bacc("TRN2", target_bir_lowering=False, debug=False,
                   enable_asserts=False, num_devices=num_devices)
    t = {
        "x": nc.dram_tensor("x", [ROWS, C], f32, kind="ExternalInput").ap(),
        "w1a": nc.dram_tensor("w1a", [C, FSH], bf16, kind="ExternalInput").ap(),
        "w2a": nc.dram_tensor("w2a", [FSH, C], bf16, kind="ExternalInput").ap(),
        "w1b": nc.dram_tensor("w1b", [C, FSH], bf16, kind="ExternalInput").ap(),
        "w2b": nc.dram_tensor("w2b", [FSH, C], bf16, kind="ExternalInput").ap(),
        "g1a": nc.dram_tensor("g1a", [FSH], f32, kind="ExternalInput").ap(),
        "b1a": nc.dram_tensor("b1a", [FSH], f32, kind="ExternalInput").ap(),
        "g1b": nc.dram_tensor("g1b", [FSH], f32, kind="ExternalInput").ap(),
        "b1b": nc.dram_tensor("b1b", [FSH], f32, kind="ExternalInput").ap(),
        "sel": nc.dram_tensor("sel", [128 * B_LOC, 128], bf16, kind="ExternalInput").ap(),
        "out": nc.dram_tensor("out", [ROWS, C], f32, kind="ExternalOutput").ap(),
    }
    with tile.TileContext(nc) as tc:
        _emit(nc, tc, tile, mybir, make_identity, t, collectives=collectives)
    nc.compile()
    _CACHE[key] = nc
    return nc


def _make_in_maps(x2, W1a, g1a, b1a, W2a, W1b, g1b, b1b, W2b):
    import ml_dtypes
    bf = ml_dtypes.bfloat16
    # W1 rows permuted to match the dma-transposed means layout: SBUF row
    # j*128 + p holds original input feature c = 4p + j.
    perm = (4 * np.arange(128)[None, :] + np.arange(4)[:, None]).reshape(-1)
    in_maps = []
    for c in range(NCORES):
        f0, f1 = c * FSH, (c + 1) * FSH
        # sel[h*64 + r, s, :] = 1 iff r == c*B_LOC + s  (h=0 bias, h=1 scale)
        sel = np.zeros((2, B, B_LOC, 128), np.float32)
        sel[:, c * B_LOC + np.arange(B_LOC), np.arange(B_LOC), :] = 1.0
        sel = sel.reshape(128 * B_LOC, 128)
        in_maps.append({
            "x": np.ascontiguousarray(x2[c * ROWS:(c + 1) * ROWS]),
            "w1a": np.ascontiguousarray(W1a[perm][:, f0:f1]).astype(bf),
            "w2a": np.ascontiguousarray(W2a[f0:f1, :]).astype(bf),
            "w1b": np.ascontiguousarray(W1b[perm][:, f0:f1]).astype(bf),
            "w2b": np.ascontiguousarray(W2b[f0:f1, :]).astype(bf),
            "g1a": np.ascontiguousarray(g1a[f0:f1]),
            "b1a": np.ascontiguousarray(b1a[f0:f1]),
            "g1b": np.ascontiguousarray(g1b[f0:f1]),
            "b1b": np.ascontiguousarray(b1b[f0:f1]),
            "sel": sel.astype(bf),
        })
    return in_maps


def _numpy_fallback(x2, npoint, W1a, g1a, b1a, W2a, W1b, g1b, b1b, W2b):
    n = x2.shape[0]
    b = npoint.shape[0]
    cum = np.cumsum(npoint)
    seg = np.searchsorted(cum, np.arange(n), side="right")
    counts = npoint.astype(x2.dtype)
    sums = np.zeros((b, x2.shape[1]), x2.dtype)
    np.add.at(sums, seg, x2)
    mean_f = sums / counts[:, None]

    def bn(h, g, bb):
        m = h.mean(0)
        v = h.var(0)
        return (h - m) / np.sqrt(v + EPS) * g + bb

    ha = np.maximum(bn(mean_f @ W1a, g1a, b1a), 0)
    out_mean = np.maximum(ha @ W2a, 0)
    hb = np.maximum(bn(mean_f @ W1b, g1b, b1b), 0)
    zw = np.maximum(hb @ W2b, 0)
    out_w = 1.0 / (1.0 + np.exp(-zw))
    return out_w[seg] * x2 * 0.5 + x2 * 0.75 + out_mean[seg]


def run_on_device(inputs, trace=False, **kwargs):
    """Returns (full_output, BassKernelResults)."""
    from concourse import bass_utils
    x2 = np.asarray(inputs["x2"], np.float32)
    args = {k: np.asarray(inputs[k], np.float32)
            for k in ("W1a", "g1a", "b1a", "W2a", "W1b", "g1b", "b1b", "W2b")}
    nc = _build()
    in_maps = _make_in_maps(x2, args["W1a"], args["g1a"], args["b1a"],
                            args["W2a"], args["W1b"], args["g1b"],
                            args["b1b"], args["W2b"])
    res = bass_utils.run_bass_kernel_spmd(
        nc, in_maps, core_ids=list(range(NCORES)), trace=trace, **kwargs)
    out = np.concatenate([res.results[c]["out"] for c in range(NCORES)], axis=0)
    return out, res


def bench_device(inputs, iters=10, warmup=2, chain=1):
    """Time the sharded NEFF execution with inputs pre-staged on device.

    chain=N runs the kernel N times back-to-back inside one dispatch (each
    call's output feeds the next call's x), so per-call device time can be
    separated from the ~80ms axon dispatch floor via (T(N)-T(1))/(N-1).

    Returns (times_sec_list, output). Mirrors bass2jax.run_bass_via_pjrt's
    multi-core path but without donation so the callable can be re-invoked.
    """
    import time
    import jax
    from jax.experimental.shard_map import shard_map
    from jax.sharding import Mesh, NamedSharding, PartitionSpec
    from concourse import bass2jax, mybir

    nc = _build()
    x2 = np.asarray(inputs["x2"], np.float32)
    args = {k: np.asarray(inputs[k], np.float32)
            for k in ("W1a", "g1a", "b1a", "W2a", "W1b", "g1b", "b1b", "W2b")}
    in_maps = _make_in_maps(x2, args["W1a"], args["g1a"], args["b1a"],
                            args["W2a"], args["W1b"], args["g1b"],
                            args["b1b"], args["W2b"])

    bass2jax.install_neuronx_cc_hook()
    partition_name = (nc.partition_id_tensor.name
                      if nc.partition_id_tensor else None)
    in_names, out_names, out_avals, zero_outs = [], [], [], []
    for alloc in nc.m.functions[0].allocations:
        if not isinstance(alloc, mybir.MemoryLocationSet):
            continue
        name = alloc.memorylocations[0].name
        if alloc.kind == "ExternalInput":
            if name != partition_name:
                in_names.append(name)
        elif alloc.kind == "ExternalOutput":
            shape = tuple(alloc.tensor_shape)
            dtype = mybir.dt.np(alloc.dtype)
            out_names.append(name)
            out_avals.append(jax.core.ShapedArray(shape, dtype))
            zero_outs.append(np.zeros(shape, dtype))
    n_params = len(in_names)
    all_in_names = list(in_names) + list(out_names)
    if partition_name is not None:
        all_in_names.append(partition_name)

    xi = in_names.index("x")

    def _body(*a):
        operands = list(a)
        if partition_name is not None:
            operands.append(bass2jax.partition_id_tensor())
        for _ in range(chain):
            outs = bass2jax._bass_exec_p.bind(
                *operands,
                out_avals=tuple(out_avals),
                in_names=tuple(all_in_names),
                out_names=tuple(out_names),
                lowering_input_output_aliases=(),
                sim_require_finite=True,
                sim_require_nnan=True,
                nc=nc,
            )
            operands[xi] = outs[0]
        return tuple(outs)

    devices = jax.devices()[:NCORES]
    mesh = Mesh(np.asarray(devices), ("core",))
    spec = PartitionSpec("core")
    n_outs = len(out_names)
    fn = jax.jit(
        shard_map(_body, mesh=mesh,
                  in_specs=(spec,) * (n_params + n_outs),
                  out_specs=(spec,) * n_outs, check_rep=False),
        keep_unused=True,
    )
    sharding = NamedSharding(mesh, spec)
    concat_in = [
        jax.device_put(
            np.concatenate([np.asarray(in_maps[c][nm]) for c in range(NCORES)],
                           axis=0), sharding)
        for nm in in_names
    ]
    concat_zero = [
        jax.device_put(np.zeros((NCORES * z.shape[0], *z.shape[1:]), z.dtype),
                       sharding)
        for z in zero_outs
    ]
    for _ in range(warmup):
        r = fn(*concat_in, *concat_zero)
        jax.block_until_ready(r)
    times = []
    for _ in range(iters):
        t0 = time.perf_counter()
        r = fn(*concat_in, *concat_zero)
        jax.block_until_ready(r)
        times.append(time.perf_counter() - t0)
    out = np.asarray(r[0]).reshape(NCORES, ROWS, C).reshape(N, C)
    return times, out


def kernel(**inputs):
    x2 = np.asarray(inputs["x2"], np.float32)
    npoint = np.asarray(inputs["npoint"])
    if (x2.shape != (N, C) or npoint.shape != (B,)
            or not np.all(npoint == SEG)):
        return _numpy_fallback(
            x2, npoint,
            *[np.asarray(inputs[k], np.float32)
              for k in ("W1a", "g1a", "b1a", "W2a", "W1b", "g1b", "b1b", "W2b")],
        ).astype(np.float32)
    out, _ = run_on_device(inputs)
    return out


# revision 34
# speedup vs baseline: 1.0731x; 1.0401x over previous
"""Trainium2 Bass kernel for nn_DCDLayer (ragged_sequence).

Math (see reference):
    mean_f[b]  = mean of x2 rows in segment b                    [B, C]
    ha         = relu(BN(mean_f @ W1a) )  ; out_mean = relu(ha @ W2a)
    hb         = relu(BN(mean_f @ W1b) )  ; out_w    = sigmoid(relu(hb @ W2b))
    out[j]     = x2[j] * (0.5*out_w[seg j] + 0.75) + out_mean[seg j]

Sharding: 8 cores, each owns 8 whole segments (32768 contiguous rows of x2).

The kernel is HBM-bound: it must read x2 (64 MiB/core) and write out
(64 MiB/core); the only tunable traffic is re-materializing x2 for the
combine after the globally-coupled BN stats.  Strategy: during the phase-A
read every x2 tile is downcast to bf16 (rel err ~1e-3, tolerance 2e-2).
NRES tiles stay resident in SBUF; the rest spill to a bf16 DRAM scratch
(half the bytes of an fp32 re-read) and are re-loaded in phase C.

Per-core flow:
  phase A: DMA fp32 tile -> ACT downcast to bf16 -> PE colsum (bf16 matmul
           vs ones into PSUM fp32, accumulated per segment) -> spill-write
           the non-resident bf16 tiles.
  AllGather means [8,512] -> [64,512]  (BatchNorm couples all segments)
  MLP feature-sharded 8-ways (256-wide slice of MID per core, sliced on the
  host into its in_map, weights pre-cast to bf16), BN stats per-feature so
  they stay local; partial second matmuls AllReduce'd ([1024,64], tiny).
  phase C: out = x_bf16 * scale_bc[seg] + bias_bc[seg]
           (DVE mul + Pool add per tile), resident tiles first, then the
           spilled tiles re-loaded from the bf16 scratch.
"""

import sys
import numpy as np

for _p in ("/opt/trn_rl_repo",):
    if _p not in sys.path:
        sys.path.insert(0, _p)

B = 64            # segments
SEG = 4096        # rows per segment
N = B * SEG
C = 512
MID = 2048
EPS = 1e-5

NCORES = 8
B_LOC = B // NCORES          # 8 segments per core
ROWS = N // NCORES           # 32768 rows per core
FSH = MID // NCORES          # 256 features of MID per core
TPB = 4                      # 128-row tiles per DMA block (1 MiB fp32 blocks)
BLK_PER_SEG = SEG // (128 * TPB)   # 8 blocks per segment
NBLK = ROWS // (128 * TPB)   # 64 blocks per core
NRES = 33                    # bf16 tiles kept resident in SBUF
NSPILL = NBLK - NRES         # bf16 tiles spilled to DRAM scratch

_CACHE = {}


def _emit(nc, tc, tile, mybir, make_identity, t, collectives=True):
    f32 = mybir.dt.float32
    f32r = mybir.dt.float32r
    bf16 = mybir.dt.bfloat16
    Alu = mybir.AluOpType
    Act = mybir.ActivationFunctionType
    X = mybir.AxisListType.X
    RG = [list(range(NCORES))]

    from contextlib import ExitStack
    ctx = ExitStack()
    consts = ctx.enter_context(tc.tile_pool(name="consts", bufs=1))
    wpool = ctx.enter_context(tc.tile_pool(name="wpool", bufs=1))
    mlp = ctx.enter_context(tc.tile_pool(name="mlp", bufs=1))
    small = ctx.enter_context(tc.tile_pool(name="small", bufs=2))
    xio = ctx.enter_context(tc.tile_pool(name="xio", bufs=3))
    xsp = ctx.enter_context(tc.tile_pool(name="xsp", bufs=4))
    resp = ctx.enter_context(tc.tile_pool(name="resp", bufs=NRES))
    bcp = ctx.enter_context(tc.tile_pool(name="bcp", bufs=B_LOC))
    psA = ctx.enter_context(tc.tile_pool(name="psA", bufs=1, space="PSUM"))
    psB = ctx.enter_context(tc.tile_pool(name="psB", bufs=3, space="PSUM"))
    psC = ctx.enter_context(tc.tile_pool(name="psC", bufs=4, space="PSUM"))
    dram = ctx.enter_context(tc.tile_pool(name="dram", bufs=1, space="DRAM"))

    # ---- constants
    ones_fr = consts.tile([128, 1], f32r)   # 1/SEG folds the mean into colsum
    nc.gpsimd.memset(ones_fr, 1.0 / SEG)
    eps_col = consts.tile([128, 1], f32)
    nc.gpsimd.memset(eps_col, EPS)
    zero_col = consts.tile([128, 1], f32)
    nc.gpsimd.memset(zero_col, 0.0)
    # preload act tables: Sigmoid first, then Sqrt, so the sqrt table (which
    # also serves Copy/Square used through phase A) is resident when the BN
    # std is computed -- no table load on the means->scale critical chain.
    warm = consts.tile([1, 1], f32)
    nc.scalar.activation(warm, zero_col[:1, :], Act.Sigmoid, bias=zero_col[:1, :])
    nc.scalar.activation(warm, zero_col[:1, :], Act.Sqrt, bias=eps_col[:1, :])

    # ---- weights (per-core feature slices, bf16 from host) -> SBUF
    def load_w(name, ap, p_tiles, fdim):
        out = []
        for k in range(p_tiles):
            w = wpool.tile([128, fdim], bf16, tag=f"{name}{k}", name=f"{name}{k}")
            nc.sync.dma_start(w, ap[k * 128:(k + 1) * 128, :])
            out.append(w)
        return out

    w1a_sb = load_w("w1a", t["w1a"], 4, FSH)   # [512,256] -> 4x[128,256]
    w1b_sb = load_w("w1b", t["w1b"], 4, FSH)
    w2a_sb = load_w("w2a", t["w2a"], 2, C)     # [256,512] -> 2x[128,512]
    w2b_sb = load_w("w2b", t["w2b"], 2, C)

    def load_gb(name, vec):   # dram [FSH] -> SBUF [128, FSH//128] (feature on partition)
        o = mlp.tile([128, FSH // 128], f32, tag=f"{name}T", name=f"{name}T")
        nc.sync.dma_start(o, vec.rearrange("(a b) -> b a", a=FSH // 128))
        return o

    gaT = load_gb("ga", t["g1a"])
    baT = load_gb("ba", t["b1a"])
    gbT = load_gb("gb", t["g1b"])
    bbT = load_gb("bb", t["b1b"])

    xv = t["x"].rearrange("(n p) c -> p n c", p=128)    # [128, 256, 512]
    ov = t["out"].rearrange("(n p) c -> p n c", p=128)

    # bf16 DRAM scratch for the NSPILL spilled tiles
    spill = dram.tile([128, NSPILL * TPB, C], bf16)

    # ---- phase A: stream fp32, downcast to bf16, PE colsum per segment
    # tiles 0..NRES-1 resident; tiles NRES..NBLK-1 spill (their writes trail
    # the last phase-A load, filling the DMA gap while the MLP runs).
    bftiles = {}
    agin = dram.tile([B_LOC, C], bf16)
    agout = dram.tile([B, C], bf16,
                      addr_space="Shared" if collectives else "Local")
    ps_seg = None
    spill_w = {}
    last_load = [None]
    for nb in range(NBLK):
        s, blk = divmod(nb, BLK_PER_SEG)
        xt = xio.tile([128, TPB, C], f32, tag="xio", name=f"xa{nb}")
        last_load[0] = nc.sync.dma_start(xt, xv[:, nb * TPB:(nb + 1) * TPB, :])
        if nb < NRES:
            xb = resp.tile([128, TPB, C], bf16, tag="res", name=f"xres{nb}")
        else:
            xb = xsp.tile([128, TPB, C], bf16, tag="xsp", name=f"xsp{nb}")
        nc.scalar.copy(xb, xt)          # ACT fp32 -> bf16
        bftiles[nb] = xb
        if blk == 0:
            ps_seg = psA.tile([1, C], f32, tag="psA", name=f"psA{s}")
        # colsum straight off the fp32 tile as f32r (1 cycle/row, and the
        # mean chain does not wait for the ACT downcast)
        for k in range(TPB):
            nc.tensor.matmul(ps_seg, lhsT=ones_fr, rhs=xt[:, k, :].bitcast(f32r),
                             start=(blk == 0 and k == 0),
                             stop=(blk == BLK_PER_SEG - 1 and k == TPB - 1))
        if nb >= NRES:
            j = nb - NRES
            # Pool-issued (SWDGE): a spill write waiting on its downcast must
            # not block later phase-A loads on the SP sequencer
            spill_w[nb] = nc.gpsimd.dma_start(
                spill[:, j * TPB:(j + 1) * TPB, :], xb)
        if blk == BLK_PER_SEG - 1:
            msr = mlp.tile([1, C], bf16, tag="msr", name=f"msr{s}")
            nc.scalar.copy(msr, ps_seg)   # psum f32 -> bf16 mean row
            nc.gpsimd.dma_start(agin[s:s + 1, :], msr)
    # keep the last few spill writes behind the final load: they fill the
    # DMA gap while the means MLP chain runs
    for nb in range(NBLK - 4, NBLK):
        tile.add_dep_helper(spill_w[nb].ins, last_load[0].ins, sync=True,
                            reason="cluster trailing spill writes in MLP gap")

    # ---- AllGather means
    if collectives:
        nc.gpsimd.collective_compute(
            "AllGather", Alu.bypass, replica_groups=RG,
            ins=[agin.opt()], outs=[agout.opt()],
        )
    else:
        nc.sync.dma_start(agout[:B_LOC, :], agin)
    # meansT via DMA xbar transpose: [64, 512] -> [512, 64] as [128, 4, 64],
    # so partition p / slot j holds feature c = 4p + j (W1 host rows are
    # permuted to match).
    mT_full = mlp.tile([128, 4, B], bf16)
    nc.sync.dma_start_transpose(mT_full, agout)
    mT = [mT_full[:, k, :] for k in range(4)]

    # ---- MLP branch: h1T = W1slice.T @ meansT ; BN per feature ; relu
    def branch(bid, w1_sb, gT, bT):
        haT = []
        for ml in range(FSH // 128):           # 2 local feature tiles
            ph = psB.tile([128, B], f32, tag="ps", name=f"ph{bid}{ml}")
            for k in range(4):
                nc.tensor.matmul(
                    ph, lhsT=w1_sb[k][:, ml * 128:(ml + 1) * 128], rhs=mT[k],
                    start=(k == 0), stop=(k == 3),
                )
            h = mlp.tile([128, B], f32, tag=f"h{bid}{ml}", name=f"h{bid}{ml}")
            s1 = small.tile([128, 1], f32, tag="s1", name=f"s1{bid}{ml}")
            nc.scalar.activation(h, ph, Act.Copy, accum_out=s1)
            sq = small.tile([128, B], f32, tag="sq", name=f"sq{bid}{ml}")
            s2 = small.tile([128, 1], f32, tag="s2", name=f"s2{bid}{ml}")
            nc.scalar.activation(sq, h, Act.Square, bias=zero_col, accum_out=s2)
            mu = small.tile([128, 1], f32, tag="mu", name=f"mu{bid}{ml}")
            nc.scalar.mul(mu, s1, 1.0 / B)
            ex2 = small.tile([128, 1], f32, tag="ex2", name=f"ex2{bid}{ml}")
            nc.scalar.mul(ex2, s2, 1.0 / B)
            # negvar = mu^2 - E[h^2]; std = sqrt(-negvar + eps)
            nv = small.tile([128, 1], f32, tag="nv", name=f"nv{bid}{ml}")
            nc.vector.scalar_tensor_tensor(nv, mu, mu, ex2,
                                           op0=Alu.mult, op1=Alu.subtract)
            std = small.tile([128, 1], f32, tag="std", name=f"std{bid}{ml}")
            nc.scalar.activation(std, nv, Act.Sqrt, bias=eps_col, scale=-1.0)
            istd = small.tile([128, 1], f32, tag="istd", name=f"istd{bid}{ml}")
            nc.vector.reciprocal(istd, std)
            sc = small.tile([128, 1], f32, tag="sc", name=f"sc{bid}{ml}")
            nc.vector.tensor_mul(sc, gT[:, ml:ml + 1], istd)
            t1 = small.tile([128, 1], f32, tag="t1", name=f"t1{bid}{ml}")
            nc.vector.tensor_mul(t1, mu, sc)
            bi = small.tile([128, 1], f32, tag="bi", name=f"bi{bid}{ml}")
            nc.vector.tensor_sub(bi, bT[:, ml:ml + 1], t1)
            ha = mlp.tile([128, B], bf16, tag=f"ha{bid}{ml}", name=f"ha{bid}{ml}")
            nc.scalar.activation(ha, h, Act.Relu, bias=bi, scale=sc)
            haT.append(ha)
        return haT

    haTa = branch("a", w1a_sb, gaT, baT)
    haTb = branch("b", w1b_sb, gbT, bbT)

    # ---- partial second matmuls in [B, C] layout (no output transposes):
    # po[b, c] = sum_mid haT[mid, b] * W2[mid, c]; branch a -> partitions
    # 0..63, branch b -> 64..127 of one PSUM tile.
    po_ps = psB.tile([128, C], f32, tag="ps", name="po_ps")
    for bi_, (w2_sb, haT) in enumerate([(w2a_sb, haTa), (w2b_sb, haTb)]):
        dst = po_ps[bi_ * B:(bi_ + 1) * B, :]
        for ml in range(FSH // 128):
            nc.tensor.matmul(
                dst, lhsT=haT[ml], rhs=w2_sb[ml],
                start=(ml == 0), stop=(ml == FSH // 128 - 1),
            )
    arin_sb = mlp.tile([128, C], bf16)
    nc.scalar.copy(arin_sb, po_ps)
    # dummy sigmoid: forces the sigmoid act-table load into the AllReduce
    # round-trip window (ACT is idle there) instead of the post chain
    nc.scalar.activation(warm, arin_sb[:1, :1], Act.Sigmoid, bias=zero_col[:1])
    arin = dram.tile([128, C], bf16)
    arout = dram.tile([128, C], bf16,
                      addr_space="Shared" if collectives else "Local")
    nc.sync.dma_start(arin, arin_sb)
    if collectives:
        nc.gpsimd.collective_compute(
            "AllReduce", Alu.add, replica_groups=RG,
            ins=[arin.opt()], outs=[arout.opt()],
        )
    else:
        nc.sync.dma_start(arout[:, :], arin)

    # ---- post-AR: bias rows stay raw (relu folds into the bbc broadcast),
    # scale rows get relu+sigmoid (the 0.5*x+0.75 affine folds into sbc).
    post_sb = mlp.tile([128, C], bf16)
    nc.sync.dma_start(post_sb, arout)
    ob = mlp.tile([128, C], bf16, tag="post_ob", name="ob")
    nc.scalar.activation(ob[B:, :], post_sb[B:, :], Act.Relu,
                         bias=zero_col[B:])
    ob2 = mlp.tile([128, C], bf16, tag="post_ob2", name="ob2")
    nc.scalar.activation(ob2[B:, :], ob[B:, :], Act.Sigmoid, bias=zero_col[B:])

    # ---- per-core replicated one-hot selector [128, 8, 128] (bf16):
    # rows 0..63 select the bias row, rows 64..127 the scale row.
    sel_sb = mlp.tile([128, B_LOC, 128], bf16)
    selv = t["sel"].rearrange("(p s) q -> p s q", s=B_LOC)
    nc.sync.dma_start(sel_sb, selv)

    # ---- per-segment broadcast scale/bias tiles, hoisted ahead of the tile
    # loop so segment boundaries never stall the phase-C pipeline (bufs=4
    # gives 4 segments of lookahead).
    # Pool (idle in phase C) does the psum->bf16 copies so they never queue
    # behind ACT's per-tile upconverts; relu/affine fold into them.
    sbcs, bbcs = [], []
    for s in range(B_LOC):
        pbs = psC.tile([128, C], f32, tag="psbc", name=f"pbs{s}")
        nc.tensor.matmul(pbs, lhsT=sel_sb[B:, s, :], rhs=ob2[B:, :],
                         start=True, stop=True)
        sbc = bcp.tile([128, C], bf16, tag="sbc", name=f"sbc{s}")
        nc.gpsimd.tensor_scalar(sbc, pbs, 0.5, 0.75, op0=Alu.mult, op1=Alu.add)
        pbb = psC.tile([128, C], f32, tag="psbc", name=f"pbb{s}")
        nc.tensor.matmul(pbb, lhsT=sel_sb[:B, s, :], rhs=post_sb[:B, :],
                         start=True, stop=True)
        bbc = bcp.tile([128, C], bf16, tag="bbc", name=f"bbc{s}")
        nc.gpsimd.tensor_scalar_max(bbc, pbb, 0.0)
        sbcs.append(sbc)
        bbcs.append(bbc)

    # ---- phase C: out = x_bf16 * scale_bc + bias_bc
    # tile order 0..NBLK-1 = residents first, then the spilled tiles (whose
    # bf16 re-loads prefetch while residents are processed).
    # All-bf16 DVE math (2x mode) in place on the x tile, then one ACT copy
    # upconverts to the fp32 store tile: DVE ~2.3us, ACT ~1.9us per tile,
    # both inside the 2.9us per-tile DMA store time.
    for nb in range(NBLK):
        s, blk = divmod(nb, BLK_PER_SEG)
        sbc_b = sbcs[s][:, None, :].broadcast_to([128, TPB, C])
        bbc_b = bbcs[s][:, None, :].broadcast_to([128, TPB, C])
        if nb < NRES:
            xb = bftiles[nb]
        else:
            j = nb - NRES
            xb = xsp.tile([128, TPB, C], bf16, tag="xsp", name=f"xrl{nb}")
            # Pool-issued: a re-load waiting on its SBUF slot must not block
            # later stores (and vice versa) on the SP sequencer
            nc.gpsimd.dma_start(xb, spill[:, j * TPB:(j + 1) * TPB, :])
        nc.vector.tensor_mul(xb, xb, sbc_b)          # DVE bf16, in place
        nc.vector.tensor_add(xb, xb, bbc_b)          # DVE bf16, in place
        ot = xio.tile([128, TPB, C], f32, tag="xio", name=f"xo{nb}")
        nc.scalar.copy(ot, xb)                       # ACT bf16 -> f32
        nc.sync.dma_start(ov[:, nb * TPB:(nb + 1) * TPB, :], ot)

    ctx.close()


def _build(num_devices=NCORES, collectives=True):
    key = ("nc", num_devices, collectives)
    if key in _CACHE:
        return _CACHE[key]
    import concourse.bacc as bacc
    import concourse.tile as tile
    from concourse import mybir
    from concourse.masks import make_identity

    f32 = mybir.dt.float32
    bf16 = mybir.dt.bfloat16
    nc = bacc.Bacc("TRN2", target_bir_lowering=False, debug=False,
                   enable_asserts=False, num_devices=num_devices)
    t = {
        "x": nc.dram_tensor("x", [ROWS, C], f32, kind="ExternalInput").ap(),
        "w1a": nc.dram_tensor("w1a", [C, FSH], bf16, kind="ExternalInput").ap(),
        "w2a": nc.dram_tensor("w2a", [FSH, C], bf16, kind="ExternalInput").ap(),
        "w1b": nc.dram_tensor("w1b", [C, FSH], bf16, kind="ExternalInput").ap(),
        "w2b": nc.dram_tensor("w2b", [FSH, C], bf16, kind="ExternalInput").ap(),
        "g1a": nc.dram_tensor("g1a", [FSH], f32, kind="ExternalInput").ap(),
        "b1a": nc.dram_tensor("b1a", [FSH], f32, kind="ExternalInput").ap(),
        "g1b": nc.dram_tensor("g1b", [FSH], f32, kind="ExternalInput").ap(),
        "b1b": nc.dram_tensor("b1b", [FSH], f32, kind="ExternalInput").ap(),
        "sel": nc.dram_tensor("sel", [128 * B_LOC, 128], bf16, kind="ExternalInput").ap(),
        "out": nc.dram_tensor("out", [ROWS, C], f32, kind="ExternalOutput").ap(),
    }
    with tile.TileContext(nc) as tc:
        _emit(nc, tc, tile, mybir, make_identity, t, collectives=collectives)
    nc.compile()
    _CACHE[key] = nc
    return nc


def _make_in_maps(x2, W1a, g1a, b1a, W2a, W1b, g1b, b1b, W2b):
    import ml_dtypes
    bf = ml_dtypes.bfloat16
    # W1 rows permuted to match the dma-transposed means layout: SBUF row
    # j*128 + p holds original input feature c = 4p + j.
    perm = (4 * np.arange(128)[None, :] + np.arange(4)[:, None]).reshape(-1)
    in_maps = []
    for c in range(NCORES):
        f0, f1 = c * FSH, (c + 1) * FSH
        # sel[h*64 + r, s, :] = 1 iff r == c*B_LOC + s  (h=0 bias, h=1 scale)
        sel = np.zeros((2, B, B_LOC, 128), np.float32)
        sel[:, c * B_LOC + np.arange(B_LOC), np.arange(B_LOC), :] = 1.0
        sel = sel.reshape(128 * B_LOC, 128)
        in_maps.append({
            "x": np.ascontiguousarray(x2[c * ROWS:(c + 1) * ROWS]),
            "w1a": np.ascontiguousarray(W1a[perm][:, f0:f1]).astype(bf),
            "w2a": np.ascontiguousarray(W2a[f0:f1, :]).astype(bf),
            "w1b": np.ascontiguousarray(W1b[perm][:, f0:f1]).astype(bf),
            "w2b": np.ascontiguousarray(W2b[f0:f1, :]).astype(bf),
            "g1a": np.ascontiguousarray(g1a[f0:f1]),
            "b1a": np.ascontiguousarray(b1a[f0:f1]),
            "g1b": np.ascontiguousarray(g1b[f0:f1]),
            "b1b": np.ascontiguousarray(b1b[f0:f1]),
            "sel": sel.astype(bf),
        })
    return in_maps


def _numpy_fallback(x2, npoint, W1a, g1a, b1a, W2a, W1b, g1b, b1b, W2b):
    n = x2.shape[0]
    b = npoint.shape[0]
    cum = np.cumsum(npoint)
    seg = np.searchsorted(cum, np.arange(n), side="right")
    counts = npoint.astype(x2.dtype)
    sums = np.zeros((b, x2.shape[1]), x2.dtype)
    np.add.at(sums, seg, x2)
    mean_f = sums / counts[:, None]

    def bn(h, g, bb):
        m = h.mean(0)
        v = h.var(0)
        return (h - m) / np.sqrt(v + EPS) * g + bb

    ha = np.maximum(bn(mean_f @ W1a, g1a, b1a), 0)
    out_mean = np.maximum(ha @ W2a, 0)
    hb = np.maximum(bn(mean_f @ W1b, g1b, b1b), 0)
    zw = np.maximum(hb @ W2b, 0)
    out_w = 1.0 / (1.0 + np.exp(-zw))
    return out_w[seg] * x2 * 0.5 + x2 * 0.75 + out_mean[seg]


def run_on_device(inputs, trace=False, **kwargs):
    """Returns (full_output, BassKernelResults)."""
    from concourse import bass_utils
    x2 = np.asarray(inputs["x2"], np.float32)
    args = {k: np.asarray(inputs[k], np.float32)
            for k in ("W1a", "g1a", "b1a", "W2a", "W1b", "g1b", "b1b", "W2b")}
    nc = _build()
    in_maps = _make_in_maps(x2, args["W1a"], args["g1a"], args["b1a"],
                            args["W2a"], args["W1b"], args["g1b"],
                            args["b1b"], args["W2b"])
    res = bass_utils.run_bass_kernel_spmd(
        nc, in_maps, core_ids=list(range(NCORES)), trace=trace, **kwargs)
    out = np.concatenate([res.results[c]["out"] for c in range(NCORES)], axis=0)
    return out, res


def bench_device(inputs, iters=10, warmup=2, chain=1):
    """Time the sharded NEFF execution with inputs pre-staged on device.

    chain=N runs the kernel N times back-to-back inside one dispatch (each
    call's output feeds the next call's x), so per-call device time can be
    separated from the ~80ms axon dispatch floor via (T(N)-T(1))/(N-1).

    Returns (times_sec_list, output). Mirrors bass2jax.run_bass_via_pjrt's
    multi-core path but without donation so the callable can be re-invoked.
    """
    import time
    import jax
    from jax.experimental.shard_map import shard_map
    from jax.sharding import Mesh, NamedSharding, PartitionSpec
    from concourse import bass2jax, mybir

    nc = _build()
    x2 = np.asarray(inputs["x2"], np.float32)
    args = {k: np.asarray(inputs[k], np.float32)
            for k in ("W1a", "g1a", "b1a", "W2a", "W1b", "g1b", "b1b", "W2b")}
    in_maps = _make_in_maps(x2, args["W1a"], args["g1a"], args["b1a"],
                            args["W2a"], args["W1b"], args["g1b"],
                            args["b1b"], args["W2b"])

    bass2jax.install_neuronx_cc_hook()
    partition_name = (nc.partition_id_tensor.name
                      if nc.partition_id_tensor else None)
    in_names, out_names, out_avals, zero_outs = [], [], [], []
    for alloc in nc.m.functions[0].allocations:
        if not isinstance(alloc, mybir.MemoryLocationSet):
            continue
        name = alloc.memorylocations[0].name
        if alloc.kind == "ExternalInput":
            if name != partition_name:
                in_names.append(name)
        elif alloc.kind == "ExternalOutput":
            shape = tuple(alloc.tensor_shape)
            dtype = mybir.dt.np(alloc.dtype)
            out_names.append(name)
            out_avals.append(jax.core.ShapedArray(shape, dtype))
            zero_outs.append(np.zeros(shape, dtype))
    n_params = len(in_names)
    all_in_names = list(in_names) + list(out_names)
    if partition_name is not None:
        all_in_names.append(partition_name)

    xi = in_names.index("x")

    def _body(*a):
        operands = list(a)
        if partition_name is not None:
            operands.append(bass2jax.partition_id_tensor())
        for _ in range(chain):
            outs = bass2jax._bass_exec_p.bind(
                *operands,
                out_avals=tuple(out_avals),
                in_names=tuple(all_in_names),
                out_names=tuple(out_names),
                lowering_input_output_aliases=(),
                sim_require_finite=True,
                sim_require_nnan=True,
                nc=nc,
            )
            operands[xi] = outs[0]
        return tuple(outs)

    devices = jax.devices()[:NCORES]
    mesh = Mesh(np.asarray(devices), ("core",))
    spec = PartitionSpec("core")
    n_outs = len(out_names)
    fn = jax.jit(
        shard_map(_body, mesh=mesh,
                  in_specs=(spec,) * (n_params + n_outs),
                  out_specs=(spec,) * n_outs, check_rep=False),
        keep_unused=True,
    )
    sharding = NamedSharding(mesh, spec)
    concat_in = [
        jax.device_put(
            np.concatenate([np.asarray(in_maps[c][nm]) for c in range(NCORES)],
                           axis=0), sharding)
        for nm in in_names
    ]
    concat_zero = [
        jax.device_put(np.zeros((NCORES * z.shape[0], *z.shape[1:]), z.dtype),
                       sharding)
        for z in zero_outs
    ]
    for _ in range(warmup):
        r = fn(*concat_in, *concat_zero)
        jax.block_until_ready(r)
    times = []
    for _ in range(iters):
        t0 = time.perf_counter()
        r = fn(*concat_in, *concat_zero)
        jax.block_until_ready(r)
        times.append(time.perf_counter() - t0)
    out = np.asarray(r[0]).reshape(NCORES, ROWS, C).reshape(N, C)
    return times, out


def kernel(**inputs):
    x2 = np.asarray(inputs["x2"], np.float32)
    npoint = np.asarray(inputs["npoint"])
    if (x2.shape != (N, C) or npoint.shape != (B,)
            or not np.all(npoint == SEG)):
        return _numpy_fallback(
            x2, npoint,
            *[np.asarray(inputs[k], np.float32)
              for k in ("W1a", "g1a", "b1a", "W2a", "W1b", "g1b", "b1b", "W2b")],
        ).astype(np.float32)
    out, _ = run_on_device(inputs)
    return out


# revision 37
# speedup vs baseline: 1.0868x; 1.0128x over previous
"""Trainium2 Bass kernel for nn_DCDLayer (ragged_sequence).

Math (see reference):
    mean_f[b]  = mean of x2 rows in segment b                    [B, C]
    ha         = relu(BN(mean_f @ W1a) )  ; out_mean = relu(ha @ W2a)
    hb         = relu(BN(mean_f @ W1b) )  ; out_w    = sigmoid(relu(hb @ W2b))
    out[j]     = x2[j] * (0.5*out_w[seg j] + 0.75) + out_mean[seg j]

Sharding: 8 cores, each owns 8 whole segments (32768 contiguous rows of x2).

The kernel is HBM-bound: it must read x2 (64 MiB/core) and write out
(64 MiB/core); the only tunable traffic is re-materializing x2 for the
combine after the globally-coupled BN stats.  Strategy: during the phase-A
read every x2 tile is downcast to bf16 (rel err ~1e-3, tolerance 2e-2).
NRES tiles stay resident in SBUF; the rest spill to a bf16 DRAM scratch
(half the bytes of an fp32 re-read) and are re-loaded in phase C.

Per-core flow:
  phase A: DMA fp32 tile -> ACT downcast to bf16 -> PE colsum (bf16 matmul
           vs ones into PSUM fp32, accumulated per segment) -> spill-write
           the non-resident bf16 tiles.
  AllGather means [8,512] -> [64,512]  (BatchNorm couples all segments)
  MLP feature-sharded 8-ways (256-wide slice of MID per core, sliced on the
  host into its in_map, weights pre-cast to bf16), BN stats per-feature so
  they stay local; partial second matmuls AllReduce'd ([1024,64], tiny).
  phase C: out = x_bf16 * scale_bc[seg] + bias_bc[seg]
           (DVE mul + Pool add per tile), resident tiles first, then the
           spilled tiles re-loaded from the bf16 scratch.
"""

import sys
import numpy as np

for _p in ("/opt/trn_rl_repo",):
    if _p not in sys.path:
        sys.path.insert(0, _p)

B = 64            # segments
SEG = 4096        # rows per segment
N = B * SEG
C = 512
MID = 2048
EPS = 1e-5

NCORES = 8
B_LOC = B // NCORES          # 8 segments per core
ROWS = N // NCORES           # 32768 rows per core
FSH = MID // NCORES          # 256 features of MID per core
TPB = 4                      # 128-row tiles per DMA block (1 MiB fp32 blocks)
BLK_PER_SEG = SEG // (128 * TPB)   # 8 blocks per segment
NBLK = ROWS // (128 * TPB)   # 64 blocks per core
NRES = 35                    # bf16 tiles kept resident in SBUF
NSPILL = NBLK - NRES         # bf16 tiles spilled to DRAM scratch

_CACHE = {}


def _emit(nc, tc, tile, mybir, make_identity, t, collectives=True):
    f32 = mybir.dt.float32
    f32r = mybir.dt.float32r
    bf16 = mybir.dt.bfloat16
    Alu = mybir.AluOpType
    Act = mybir.ActivationFunctionType
    X = mybir.AxisListType.X
    RG = [list(range(NCORES))]

    from contextlib import ExitStack
    ctx = ExitStack()
    consts = ctx.enter_context(tc.tile_pool(name="consts", bufs=1))
    wpool = ctx.enter_context(tc.tile_pool(name="wpool", bufs=1))
    mlp = ctx.enter_context(tc.tile_pool(name="mlp", bufs=1))
    small = ctx.enter_context(tc.tile_pool(name="small", bufs=2))
    xio = ctx.enter_context(tc.tile_pool(name="xio", bufs=3))
    xsp = ctx.enter_context(tc.tile_pool(name="xsp", bufs=4))
    resp = ctx.enter_context(tc.tile_pool(name="resp", bufs=NRES))
    bcp = ctx.enter_context(tc.tile_pool(name="bcp", bufs=4))
    psA = ctx.enter_context(tc.tile_pool(name="psA", bufs=1, space="PSUM"))
    psB = ctx.enter_context(tc.tile_pool(name="psB", bufs=3, space="PSUM"))
    psC = ctx.enter_context(tc.tile_pool(name="psC", bufs=4, space="PSUM"))
    dram = ctx.enter_context(tc.tile_pool(name="dram", bufs=1, space="DRAM"))

    # ---- constants
    ones_fr = consts.tile([128, 1], f32r)   # 1/SEG folds the mean into colsum
    nc.gpsimd.memset(ones_fr, 1.0 / SEG)
    eps_col = consts.tile([128, 1], f32)
    nc.gpsimd.memset(eps_col, EPS)
    zero_col = consts.tile([128, 1], f32)
    nc.gpsimd.memset(zero_col, 0.0)
    # preload act tables: Sigmoid first, then Sqrt, so the sqrt table (which
    # also serves Copy/Square used through phase A) is resident when the BN
    # std is computed -- no table load on the means->scale critical chain.
    warm = consts.tile([1, 1], f32)
    nc.scalar.activation(warm, zero_col[:1, :], Act.Sigmoid, bias=zero_col[:1, :])
    nc.scalar.activation(warm, zero_col[:1, :], Act.Sqrt, bias=eps_col[:1, :])

    # ---- weights (per-core feature slices, bf16 from host) -> SBUF
    def load_w(name, ap, p_tiles, fdim):
        out = []
        for k in range(p_tiles):
            w = wpool.tile([128, fdim], bf16, tag=f"{name}{k}", name=f"{name}{k}")
            nc.sync.dma_start(w, ap[k * 128:(k + 1) * 128, :])
            out.append(w)
        return out

    w1a_sb = load_w("w1a", t["w1a"], 4, FSH)   # [512,256] -> 4x[128,256]
    w1b_sb = load_w("w1b", t["w1b"], 4, FSH)
    w2a_sb = load_w("w2a", t["w2a"], 2, C)     # [256,512] -> 2x[128,512]
    w2b_sb = load_w("w2b", t["w2b"], 2, C)

    def load_gb(name, vec):   # dram [FSH] -> SBUF [128, FSH//128] (feature on partition)
        o = mlp.tile([128, FSH // 128], f32, tag=f"{name}T", name=f"{name}T")
        nc.sync.dma_start(o, vec.rearrange("(a b) -> b a", a=FSH // 128))
        return o

    gaT = load_gb("ga", t["g1a"])
    baT = load_gb("ba", t["b1a"])
    gbT = load_gb("gb", t["g1b"])
    bbT = load_gb("bb", t["b1b"])

    xv = t["x"].rearrange("(n p) c -> p n c", p=128)    # [128, 256, 512]
    ov = t["out"].rearrange("(n p) c -> p n c", p=128)

    # bf16 DRAM scratch for the NSPILL spilled tiles
    spill = dram.tile([128, NSPILL * TPB, C], bf16)

    # ---- phase A: stream fp32, downcast to bf16, PE colsum per segment
    # tiles 0..NRES-1 resident; tiles NRES..NBLK-1 spill (their writes trail
    # the last phase-A load, filling the DMA gap while the MLP runs).
    bftiles = {}
    agin = dram.tile([B_LOC, C], bf16)
    agout = dram.tile([B, C], bf16,
                      addr_space="Shared" if collectives else "Local")
    ps_seg = None
    spill_w = {}
    last_load = [None]
    for nb in range(NBLK):
        s, blk = divmod(nb, BLK_PER_SEG)
        xt = xio.tile([128, TPB, C], f32, tag="xio", name=f"xa{nb}")
        last_load[0] = nc.sync.dma_start(xt, xv[:, nb * TPB:(nb + 1) * TPB, :])
        if nb < NRES:
            xb = resp.tile([128, TPB, C], bf16, tag="res", name=f"xres{nb}")
        else:
            xb = xsp.tile([128, TPB, C], bf16, tag="xsp", name=f"xsp{nb}")
        nc.scalar.copy(xb, xt)          # ACT fp32 -> bf16
        bftiles[nb] = xb
        if blk == 0:
            ps_seg = psA.tile([1, C], f32, tag="psA", name=f"psA{s}")
        # colsum straight off the fp32 tile as f32r (1 cycle/row, and the
        # mean chain does not wait for the ACT downcast)
        for k in range(TPB):
            nc.tensor.matmul(ps_seg, lhsT=ones_fr, rhs=xt[:, k, :].bitcast(f32r),
                             start=(blk == 0 and k == 0),
                             stop=(blk == BLK_PER_SEG - 1 and k == TPB - 1))
        if nb >= NRES:
            j = nb - NRES
            # Pool-issued (SWDGE): a spill write waiting on its downcast must
            # not block later phase-A loads on the SP sequencer
            spill_w[nb] = nc.gpsimd.dma_start(
                spill[:, j * TPB:(j + 1) * TPB, :], xb)
        if blk == BLK_PER_SEG - 1:
            msr = mlp.tile([1, C], bf16, tag="msr", name=f"msr{s}")
            nc.scalar.copy(msr, ps_seg)   # psum f32 -> bf16 mean row
            nc.gpsimd.dma_start(agin[s:s + 1, :], msr)
    # keep the last few spill writes behind the final load: they fill the
    # DMA gap while the means MLP chain runs
    for nb in range(NBLK - 4, NBLK):
        tile.add_dep_helper(spill_w[nb].ins, last_load[0].ins, sync=True,
                            reason="cluster trailing spill writes in MLP gap")

    # ---- AllGather means
    if collectives:
        nc.gpsimd.collective_compute(
            "AllGather", Alu.bypass, replica_groups=RG,
            ins=[agin.opt()], outs=[agout.opt()],
        )
    else:
        nc.sync.dma_start(agout[:B_LOC, :], agin)
    # meansT via DMA xbar transpose: [64, 512] -> [512, 64] as [128, 4, 64],
    # so partition p / slot j holds feature c = 4p + j (W1 host rows are
    # permuted to match).
    mT_full = mlp.tile([128, 4, B], bf16)
    nc.sync.dma_start_transpose(mT_full, agout)
    mT = [mT_full[:, k, :] for k in range(4)]

    # ---- MLP branch: h1T = W1slice.T @ meansT ; BN per feature ; relu
    def branch(bid, w1_sb, gT, bT):
        haT = []
        for ml in range(FSH // 128):           # 2 local feature tiles
            ph = psB.tile([128, B], f32, tag="ps", name=f"ph{bid}{ml}")
            for k in range(4):
                nc.tensor.matmul(
                    ph, lhsT=w1_sb[k][:, ml * 128:(ml + 1) * 128], rhs=mT[k],
                    start=(k == 0), stop=(k == 3),
                )
            h = mlp.tile([128, B], f32, tag=f"h{bid}{ml}", name=f"h{bid}{ml}")
            s1 = small.tile([128, 1], f32, tag="s1", name=f"s1{bid}{ml}")
            nc.scalar.activation(h, ph, Act.Copy, accum_out=s1)
            sq = small.tile([128, B], f32, tag="sq", name=f"sq{bid}{ml}")
            s2 = small.tile([128, 1], f32, tag="s2", name=f"s2{bid}{ml}")
            nc.scalar.activation(sq, h, Act.Square, bias=zero_col, accum_out=s2)
            mu = small.tile([128, 1], f32, tag="mu", name=f"mu{bid}{ml}")
            nc.scalar.mul(mu, s1, 1.0 / B)
            ex2 = small.tile([128, 1], f32, tag="ex2", name=f"ex2{bid}{ml}")
            nc.scalar.mul(ex2, s2, 1.0 / B)
            # negvar = mu^2 - E[h^2]; std = sqrt(-negvar + eps)
            nv = small.tile([128, 1], f32, tag="nv", name=f"nv{bid}{ml}")
            nc.vector.scalar_tensor_tensor(nv, mu, mu, ex2,
                                           op0=Alu.mult, op1=Alu.subtract)
            std = small.tile([128, 1], f32, tag="std", name=f"std{bid}{ml}")
            nc.scalar.activation(std, nv, Act.Sqrt, bias=eps_col, scale=-1.0)
            istd = small.tile([128, 1], f32, tag="istd", name=f"istd{bid}{ml}")
            nc.vector.reciprocal(istd, std)
            sc = small.tile([128, 1], f32, tag="sc", name=f"sc{bid}{ml}")
            nc.vector.tensor_mul(sc, gT[:, ml:ml + 1], istd)
            t1 = small.tile([128, 1], f32, tag="t1", name=f"t1{bid}{ml}")
            nc.vector.tensor_mul(t1, mu, sc)
            bi = small.tile([128, 1], f32, tag="bi", name=f"bi{bid}{ml}")
            nc.vector.tensor_sub(bi, bT[:, ml:ml + 1], t1)
            ha = mlp.tile([128, B], bf16, tag=f"ha{bid}{ml}", name=f"ha{bid}{ml}")
            nc.scalar.activation(ha, h, Act.Relu, bias=bi, scale=sc)
            haT.append(ha)
        return haT

    haTa = branch("a", w1a_sb, gaT, baT)
    haTb = branch("b", w1b_sb, gbT, bbT)

    # ---- partial second matmuls in [B, C] layout (no output transposes):
    # po[b, c] = sum_mid haT[mid, b] * W2[mid, c]; branch a -> partitions
    # 0..63, branch b -> 64..127 of one PSUM tile.
    po_ps = psB.tile([128, C], f32, tag="ps", name="po_ps")
    for bi_, (w2_sb, haT) in enumerate([(w2a_sb, haTa), (w2b_sb, haTb)]):
        dst = po_ps[bi_ * B:(bi_ + 1) * B, :]
        for ml in range(FSH // 128):
            nc.tensor.matmul(
                dst, lhsT=haT[ml], rhs=w2_sb[ml],
                start=(ml == 0), stop=(ml == FSH // 128 - 1),
            )
    arin_sb = mlp.tile([128, C], bf16)
    nc.scalar.copy(arin_sb, po_ps)
    # dummy sigmoid: forces the sigmoid act-table load into the AllReduce
    # round-trip window (ACT is idle there) instead of the post chain
    nc.scalar.activation(warm, arin_sb[:1, :1], Act.Sigmoid, bias=zero_col[:1])
    arin = dram.tile([128, C], bf16)
    arout = dram.tile([128, C], bf16,
                      addr_space="Shared" if collectives else "Local")
    nc.sync.dma_start(arin, arin_sb)
    if collectives:
        nc.gpsimd.collective_compute(
            "AllReduce", Alu.add, replica_groups=RG,
            ins=[arin.opt()], outs=[arout.opt()],
        )
    else:
        nc.sync.dma_start(arout[:, :], arin)

    # ---- post-AR: bias rows stay raw (relu folds into the bbc broadcast),
    # scale rows get relu+sigmoid (the 0.5*x+0.75 affine folds into sbc).
    post_sb = mlp.tile([128, C], bf16)
    nc.sync.dma_start(post_sb, arout)
    ob = mlp.tile([128, C], bf16, tag="post_ob", name="ob")
    nc.scalar.activation(ob[B:, :], post_sb[B:, :], Act.Relu,
                         bias=zero_col[B:])
    ob2 = mlp.tile([128, C], bf16, tag="post_ob2", name="ob2")
    nc.scalar.activation(ob2[B:, :], ob[B:, :], Act.Sigmoid, bias=zero_col[B:])

    # ---- per-core replicated one-hot selector [128, 8, 128] (bf16):
    # rows 0..63 select the bias row, rows 64..127 the scale row.
    sel_sb = mlp.tile([128, B_LOC, 128], bf16)
    selv = t["sel"].rearrange("(p s) q -> p s q", s=B_LOC)
    nc.sync.dma_start(sel_sb, selv)

    # ---- per-segment broadcast scale/bias tiles, hoisted ahead of the tile
    # loop so segment boundaries never stall the phase-C pipeline (bufs=4
    # gives 4 segments of lookahead).
    # Pool (idle in phase C) does the psum->bf16 copies so they never queue
    # behind ACT's per-tile upconverts; relu/affine fold into them.
    sbcs, bbcs = [], []
    for s in range(B_LOC):
        pbs = psC.tile([128, C], f32, tag="psbc", name=f"pbs{s}")
        nc.tensor.matmul(pbs, lhsT=sel_sb[B:, s, :], rhs=ob2[B:, :],
                         start=True, stop=True)
        sbc = bcp.tile([128, C], bf16, tag="sbc", name=f"sbc{s}")
        nc.gpsimd.tensor_scalar(sbc, pbs, 0.5, 0.75, op0=Alu.mult, op1=Alu.add)
        pbb = psC.tile([128, C], f32, tag="psbc", name=f"pbb{s}")
        nc.tensor.matmul(pbb, lhsT=sel_sb[:B, s, :], rhs=post_sb[:B, :],
                         start=True, stop=True)
        bbc = bcp.tile([128, C], bf16, tag="bbc", name=f"bbc{s}")
        nc.gpsimd.tensor_scalar_max(bbc, pbb, 0.0)
        sbcs.append(sbc)
        bbcs.append(bbc)

    # ---- phase C: out = x_bf16 * scale_bc + bias_bc
    # tile order 0..NBLK-1 = residents first, then the spilled tiles (whose
    # bf16 re-loads prefetch while residents are processed).
    # All-bf16 DVE math (2x mode) in place on the x tile, then one ACT copy
    # upconverts to the fp32 store tile: DVE ~2.3us, ACT ~1.9us per tile,
    # both inside the 2.9us per-tile DMA store time.
    for nb in range(NBLK):
        s, blk = divmod(nb, BLK_PER_SEG)
        sbc_b = sbcs[s][:, None, :].broadcast_to([128, TPB, C])
        bbc_b = bbcs[s][:, None, :].broadcast_to([128, TPB, C])
        if nb < NRES:
            xb = bftiles[nb]
        else:
            j = nb - NRES
            xb = xsp.tile([128, TPB, C], bf16, tag="xsp", name=f"xrl{nb}")
            # Pool-issued: a re-load waiting on its SBUF slot must not block
            # later stores (and vice versa) on the SP sequencer
            nc.gpsimd.dma_start(xb, spill[:, j * TPB:(j + 1) * TPB, :])
        ot = xio.tile([128, TPB, C], f32, tag="xio", name=f"xo{nb}")
        if nb == 0 or nb == NBLK - 1:
            # half-tile granularity on the first/last tile shortens pipeline
            # fill after the MLP and the final drain
            Hh = TPB // 2
            for hh in range(2):
                sl = slice(hh * Hh, (hh + 1) * Hh)
                nc.vector.tensor_mul(xb[:, sl, :], xb[:, sl, :], sbc_b[:, sl, :])
                nc.vector.tensor_add(xb[:, sl, :], xb[:, sl, :], bbc_b[:, sl, :])
                nc.scalar.copy(ot[:, sl, :], xb[:, sl, :])
                nc.sync.dma_start(
                    ov[:, nb * TPB + hh * Hh:nb * TPB + (hh + 1) * Hh, :],
                    ot[:, sl, :])
        else:
            nc.vector.tensor_mul(xb, xb, sbc_b)      # DVE bf16, in place
            nc.vector.tensor_add(xb, xb, bbc_b)      # DVE bf16, in place
            nc.scalar.copy(ot, xb)                   # ACT bf16 -> f32
            nc.sync.dma_start(ov[:, nb * TPB:(nb + 1) * TPB, :], ot)

    ctx.close()


def _build(num_devices=NCORES, collectives=True):
    key = ("nc", num_devices, collectives)
    if key in _CACHE:
        return _CACHE[key]
    import concourse.bacc as bacc
    import concourse.tile as tile
    from concourse import mybir
    from concourse.masks import make_identity

    f32 = mybir.dt.float32
    bf16 = mybir.dt.bfloat16
    nc = bacc.Bacc("TRN2", target_bir_lowering=False, debug=False,
                   enable_asserts=False, num_devices=num_devices)
    t = {
        "x": nc.dram_tensor("x", [ROWS, C], f32, kind="ExternalInput").ap(),
        "w1a": nc.dram_tensor("w1a", [C, FSH], bf16, kind="ExternalInput").ap(),
        "w2a": nc.dram_tensor("w2a", [FSH, C], bf16, kind="ExternalInput").ap(),
        "w1b": nc.dram_tensor("w1b", [C, FSH], bf16, kind="ExternalInput").ap(),
        "w2b": nc.dram_tensor("w2b", [FSH, C], bf16, kind="ExternalInput").ap(),
        "g1a": nc.dram_tensor("g1a", [FSH], f32, kind="ExternalInput").ap(),
        "b1a": nc.dram_tensor("b1a", [FSH], f32, kind="ExternalInput").ap(),
        "g1b": nc.dram_tensor("g1b", [FSH], f32, kind="ExternalInput").ap(),
        "b1b": nc.dram_tensor("b1b", [FSH], f32, kind="ExternalInput").ap(),
        "sel": nc.dram_tensor("sel", [128 * B_LOC, 128], bf16, kind="ExternalInput").ap(),
        "out": nc.dram_tensor("out", [ROWS, C], f32, kind="ExternalOutput").ap(),
    }
    with tile.TileContext(nc) as tc:
        _emit(nc, tc, tile, mybir, make_identity, t, collectives=collectives)
    nc.compile()
    _CACHE[key] = nc
    return nc


def _make_in_maps(x2, W1a, g1a, b1a, W2a, W1b, g1b, b1b, W2b):
    import ml_dtypes
    bf = ml_dtypes.bfloat16
    # W1 rows permuted to match the dma-transposed means layout: SBUF row
    # j*128 + p holds original input feature c = 4p + j.
    perm = (4 * np.arange(128)[None, :] + np.arange(4)[:, None]).reshape(-1)
    in_maps = []
    for c in range(NCORES):
        f0, f1 = c * FSH, (c + 1) * FSH
        # sel[h*64 + r, s, :] = 1 iff r == c*B_LOC + s  (h=0 bias, h=1 scale)
        sel = np.zeros((2, B, B_LOC, 128), np.float32)
        sel[:, c * B_LOC + np.arange(B_LOC), np.arange(B_LOC), :] = 1.0
        sel = sel.reshape(128 * B_LOC, 128)
        in_maps.append({
            "x": np.ascontiguousarray(x2[c * ROWS:(c + 1) * ROWS]),
            "w1a": np.ascontiguousarray(W1a[perm][:, f0:f1]).astype(bf),
            "w2a": np.ascontiguousarray(W2a[f0:f1, :]).astype(bf),
            "w1b": np.ascontiguousarray(W1b[perm][:, f0:f1]).astype(bf),
            "w2b": np.ascontiguousarray(W2b[f0:f1, :]).astype(bf),
            "g1a": np.ascontiguousarray(g1a[f0:f1]),
            "b1a": np.ascontiguousarray(b1a[f0:f1]),
            "g1b": np.ascontiguousarray(g1b[f0:f1]),
            "b1b": np.ascontiguousarray(b1b[f0:f1]),
            "sel": sel.astype(bf),
        })
    return in_maps


def _numpy_fallback(x2, npoint, W1a, g1a, b1a, W2a, W1b, g1b, b1b, W2b):
    n = x2.shape[0]
    b = npoint.shape[0]
    cum = np.cumsum(npoint)
    seg = np.searchsorted(cum, np.arange(n), side="right")
    counts = npoint.astype(x2.dtype)
    sums = np.zeros((b, x2.shape[1]), x2.dtype)
    np.add.at(sums, seg, x2)
    mean_f = sums / counts[:, None]

    def bn(h, g, bb):
        m = h.mean(0)
        v = h.var(0)
        return (h - m) / np.sqrt(v + EPS) * g + bb

    ha = np.maximum(bn(mean_f @ W1a, g1a, b1a), 0)
    out_mean = np.maximum(ha @ W2a, 0)
    hb = np.maximum(bn(mean_f @ W1b, g1b, b1b), 0)
    zw = np.maximum(hb @ W2b, 0)
    out_w = 1.0 / (1.0 + np.exp(-zw))
    return out_w[seg] * x2 * 0.5 + x2 * 0.75 + out_mean[seg]


def run_on_device(inputs, trace=False, **kwargs):
    """Returns (full_output, BassKernelResults)."""
    from concourse import bass_utils
    x2 = np.asarray(inputs["x2"], np.float32)
    args = {k: np.asarray(inputs[k], np.float32)
            for k in ("W1a", "g1a", "b1a", "W2a", "W1b", "g1b", "b1b", "W2b")}
    nc = _build()
    in_maps = _make_in_maps(x2, args["W1a"], args["g1a"], args["b1a"],
                            args["W2a"], args["W1b"], args["g1b"],
                            args["b1b"], args["W2b"])
    res = bass_utils.run_bass_kernel_spmd(
        nc, in_maps, core_ids=list(range(NCORES)), trace=trace, **kwargs)
    out = np.concatenate([res.results[c]["out"] for c in range(NCORES)], axis=0)
    return out, res


def bench_device(inputs, iters=10, warmup=2, chain=1):
    """Time the sharded NEFF execution with inputs pre-staged on device.

    chain=N runs the kernel N times back-to-back inside one dispatch (each
    call's output feeds the next call's x), so per-call device time can be
    separated from the ~80ms axon dispatch floor via (T(N)-T(1))/(N-1).

    Returns (times_sec_list, output). Mirrors bass2jax.run_bass_via_pjrt's
    multi-core path but without donation so the callable can be re-invoked.
    """
    import time
    import jax
    from jax.experimental.shard_map import shard_map
    from jax.sharding import Mesh, NamedSharding, PartitionSpec
    from concourse import bass2jax, mybir

    nc = _build()
    x2 = np.asarray(inputs["x2"], np.float32)
    args = {k: np.asarray(inputs[k], np.float32)
            for k in ("W1a", "g1a", "b1a", "W2a", "W1b", "g1b", "b1b", "W2b")}
    in_maps = _make_in_maps(x2, args["W1a"], args["g1a"], args["b1a"],
                            args["W2a"], args["W1b"], args["g1b"],
                            args["b1b"], args["W2b"])

    bass2jax.install_neuronx_cc_hook()
    partition_name = (nc.partition_id_tensor.name
                      if nc.partition_id_tensor else None)
    in_names, out_names, out_avals, zero_outs = [], [], [], []
    for alloc in nc.m.functions[0].allocations:
        if not isinstance(alloc, mybir.MemoryLocationSet):
            continue
        name = alloc.memorylocations[0].name
        if alloc.kind == "ExternalInput":
            if name != partition_name:
                in_names.append(name)
        elif alloc.kind == "ExternalOutput":
            shape = tuple(alloc.tensor_shape)
            dtype = mybir.dt.np(alloc.dtype)
            out_names.append(name)
            out_avals.append(jax.core.ShapedArray(shape, dtype))
            zero_outs.append(np.zeros(shape, dtype))
    n_params = len(in_names)
    all_in_names = list(in_names) + list(out_names)
    if partition_name is not None:
        all_in_names.append(partition_name)

    xi = in_names.index("x")

    def _body(*a):
        operands = list(a)
        if partition_name is not None:
            operands.append(bass2jax.partition_id_tensor())
        for _ in range(chain):
            outs = bass2jax._bass_exec_p.bind(
                *operands,
                out_avals=tuple(out_avals),
                in_names=tuple(all_in_names),
                out_names=tuple(out_names),
                lowering_input_output_aliases=(),
                sim_require_finite=True,
                sim_require_nnan=True,
                nc=nc,
            )
            operands[xi] = outs[0]
        return tuple(outs)

    devices = jax.devices()[:NCORES]
    mesh = Mesh(np.asarray(devices), ("core",))
    spec = PartitionSpec("core")
    n_outs = len(out_names)
    fn = jax.jit(
        shard_map(_body, mesh=mesh,
                  in_specs=(spec,) * (n_params + n_outs),
                  out_specs=(spec,) * n_outs, check_rep=False),
        keep_unused=True,
    )
    sharding = NamedSharding(mesh, spec)
    concat_in = [
        jax.device_put(
            np.concatenate([np.asarray(in_maps[c][nm]) for c in range(NCORES)],
                           axis=0), sharding)
        for nm in in_names
    ]
    concat_zero = [
        jax.device_put(np.zeros((NCORES * z.shape[0], *z.shape[1:]), z.dtype),
                       sharding)
        for z in zero_outs
    ]
    for _ in range(warmup):
        r = fn(*concat_in, *concat_zero)
        jax.block_until_ready(r)
    times = []
    for _ in range(iters):
        t0 = time.perf_counter()
        r = fn(*concat_in, *concat_zero)
        jax.block_until_ready(r)
        times.append(time.perf_counter() - t0)
    out = np.asarray(r[0]).reshape(NCORES, ROWS, C).reshape(N, C)
    return times, out


def kernel(**inputs):
    x2 = np.asarray(inputs["x2"], np.float32)
    npoint = np.asarray(inputs["npoint"])
    if (x2.shape != (N, C) or npoint.shape != (B,)
            or not np.all(npoint == SEG)):
        return _numpy_fallback(
            x2, npoint,
            *[np.asarray(inputs[k], np.float32)
              for k in ("W1a", "g1a", "b1a", "W2a", "W1b", "g1b", "b1b", "W2b")],
        ).astype(np.float32)
    out, _ = run_on_device(inputs)
    return out


# revision 38
# speedup vs baseline: 1.0872x; 1.0004x over previous
"""Trainium2 Bass kernel for nn_DCDLayer (ragged_sequence).

Math (see reference):
    mean_f[b]  = mean of x2 rows in segment b                    [B, C]
    ha         = relu(BN(mean_f @ W1a) )  ; out_mean = relu(ha @ W2a)
    hb         = relu(BN(mean_f @ W1b) )  ; out_w    = sigmoid(relu(hb @ W2b))
    out[j]     = x2[j] * (0.5*out_w[seg j] + 0.75) + out_mean[seg j]

Sharding: 8 cores, each owns 8 whole segments (32768 contiguous rows of x2).

The kernel is HBM-bound: it must read x2 (64 MiB/core) and write out
(64 MiB/core); the only tunable traffic is re-materializing x2 for the
combine after the globally-coupled BN stats.  Strategy: during the phase-A
read every x2 tile is downcast to bf16 (rel err ~1e-3, tolerance 2e-2).
NRES tiles stay resident in SBUF; the rest spill to a bf16 DRAM scratch
(half the bytes of an fp32 re-read) and are re-loaded in phase C.

Per-core flow:
  phase A: DMA fp32 tile -> ACT downcast to bf16 -> PE colsum (bf16 matmul
           vs ones into PSUM fp32, accumulated per segment) -> spill-write
           the non-resident bf16 tiles.
  AllGather means [8,512] -> [64,512]  (BatchNorm couples all segments)
  MLP feature-sharded 8-ways (256-wide slice of MID per core, sliced on the
  host into its in_map, weights pre-cast to bf16), BN stats per-feature so
  they stay local; partial second matmuls AllReduce'd ([1024,64], tiny).
  phase C: out = x_bf16 * scale_bc[seg] + bias_bc[seg]
           (DVE mul + Pool add per tile), resident tiles first, then the
           spilled tiles re-loaded from the bf16 scratch.
"""

import sys
import numpy as np

for _p in ("/opt/trn_rl_repo",):
    if _p not in sys.path:
        sys.path.insert(0, _p)

B = 64            # segments
SEG = 4096        # rows per segment
N = B * SEG
C = 512
MID = 2048
EPS = 1e-5

NCORES = 8
B_LOC = B // NCORES          # 8 segments per core
ROWS = N // NCORES           # 32768 rows per core
FSH = MID // NCORES          # 256 features of MID per core
TPB = 4                      # 128-row tiles per DMA block (1 MiB fp32 blocks)
BLK_PER_SEG = SEG // (128 * TPB)   # 8 blocks per segment
NBLK = ROWS // (128 * TPB)   # 64 blocks per core
NRES = 35                    # bf16 tiles kept resident in SBUF
NSPILL = NBLK - NRES         # bf16 tiles spilled to DRAM scratch

_CACHE = {}


def _emit(nc, tc, tile, mybir, make_identity, t, collectives=True):
    f32 = mybir.dt.float32
    f32r = mybir.dt.float32r
    bf16 = mybir.dt.bfloat16
    Alu = mybir.AluOpType
    Act = mybir.ActivationFunctionType
    X = mybir.AxisListType.X
    RG = [list(range(NCORES))]

    from contextlib import ExitStack
    ctx = ExitStack()
    consts = ctx.enter_context(tc.tile_pool(name="consts", bufs=1))
    wpool = ctx.enter_context(tc.tile_pool(name="wpool", bufs=1))
    mlp = ctx.enter_context(tc.tile_pool(name="mlp", bufs=1))
    small = ctx.enter_context(tc.tile_pool(name="small", bufs=2))
    xio = ctx.enter_context(tc.tile_pool(name="xio", bufs=3))
    xsp = ctx.enter_context(tc.tile_pool(name="xsp", bufs=4))
    resp = ctx.enter_context(tc.tile_pool(name="resp", bufs=NRES))
    bcp = ctx.enter_context(tc.tile_pool(name="bcp", bufs=4))
    psA = ctx.enter_context(tc.tile_pool(name="psA", bufs=1, space="PSUM"))
    psB = ctx.enter_context(tc.tile_pool(name="psB", bufs=3, space="PSUM"))
    psC = ctx.enter_context(tc.tile_pool(name="psC", bufs=4, space="PSUM"))
    dram = ctx.enter_context(tc.tile_pool(name="dram", bufs=1, space="DRAM"))

    # ---- constants
    ones_fr = consts.tile([128, 1], f32r)   # 1/SEG folds the mean into colsum
    nc.gpsimd.memset(ones_fr, 1.0 / SEG)
    eps_col = consts.tile([128, 1], f32)
    nc.gpsimd.memset(eps_col, EPS)
    zero_col = consts.tile([128, 1], f32)
    nc.gpsimd.memset(zero_col, 0.0)
    # preload act tables: Sigmoid first, then Sqrt, so the sqrt table (which
    # also serves Copy/Square used through phase A) is resident when the BN
    # std is computed -- no table load on the means->scale critical chain.
    warm = consts.tile([1, 1], f32)
    nc.scalar.activation(warm, zero_col[:1, :], Act.Sigmoid, bias=zero_col[:1, :])
    nc.scalar.activation(warm, zero_col[:1, :], Act.Sqrt, bias=eps_col[:1, :])

    # ---- weights (per-core feature slices, bf16 from host) -> SBUF
    def load_w(name, ap, p_tiles, fdim):
        out = []
        for k in range(p_tiles):
            w = wpool.tile([128, fdim], bf16, tag=f"{name}{k}", name=f"{name}{k}")
            nc.sync.dma_start(w, ap[k * 128:(k + 1) * 128, :])
            out.append(w)
        return out

    w1a_sb = load_w("w1a", t["w1a"], 4, FSH)   # [512,256] -> 4x[128,256]
    w1b_sb = load_w("w1b", t["w1b"], 4, FSH)
    w2a_sb = load_w("w2a", t["w2a"], 2, C)     # [256,512] -> 2x[128,512]
    w2b_sb = load_w("w2b", t["w2b"], 2, C)

    def load_gb(name, vec):   # dram [FSH] -> SBUF [128, FSH//128] (feature on partition)
        o = mlp.tile([128, FSH // 128], f32, tag=f"{name}T", name=f"{name}T")
        nc.sync.dma_start(o, vec.rearrange("(a b) -> b a", a=FSH // 128))
        return o

    gaT = load_gb("ga", t["g1a"])
    baT = load_gb("ba", t["b1a"])
    gbT = load_gb("gb", t["g1b"])
    bbT = load_gb("bb", t["b1b"])

    xv = t["x"].rearrange("(n p) c -> p n c", p=128)    # [128, 256, 512]
    ov = t["out"].rearrange("(n p) c -> p n c", p=128)

    # bf16 DRAM scratch for the NSPILL spilled tiles
    spill = dram.tile([128, NSPILL * TPB, C], bf16)

    # ---- phase A: stream fp32, downcast to bf16, PE colsum per segment
    # tiles 0..NRES-1 resident; tiles NRES..NBLK-1 spill (their writes trail
    # the last phase-A load, filling the DMA gap while the MLP runs).
    bftiles = {}
    agin = dram.tile([B_LOC, C], bf16)
    agout = dram.tile([B, C], bf16,
                      addr_space="Shared" if collectives else "Local")
    ps_seg = None
    spill_w = {}
    last_load = [None]
    for nb in range(NBLK):
        s, blk = divmod(nb, BLK_PER_SEG)
        xt = xio.tile([128, TPB, C], f32, tag="xio", name=f"xa{nb}")
        last_load[0] = nc.sync.dma_start(xt, xv[:, nb * TPB:(nb + 1) * TPB, :])
        if nb < NRES:
            xb = resp.tile([128, TPB, C], bf16, tag="res", name=f"xres{nb}")
        else:
            xb = xsp.tile([128, TPB, C], bf16, tag="xsp", name=f"xsp{nb}")
        nc.scalar.copy(xb, xt)          # ACT fp32 -> bf16
        bftiles[nb] = xb
        if blk == 0:
            ps_seg = psA.tile([1, C], f32, tag="psA", name=f"psA{s}")
        # colsum straight off the fp32 tile as f32r (1 cycle/row, and the
        # mean chain does not wait for the ACT downcast)
        for k in range(TPB):
            nc.tensor.matmul(ps_seg, lhsT=ones_fr, rhs=xt[:, k, :].bitcast(f32r),
                             start=(blk == 0 and k == 0),
                             stop=(blk == BLK_PER_SEG - 1 and k == TPB - 1))
        if nb >= NRES:
            j = nb - NRES
            # Pool-issued (SWDGE): a spill write waiting on its downcast must
            # not block later phase-A loads on the SP sequencer
            spill_w[nb] = nc.gpsimd.dma_start(
                spill[:, j * TPB:(j + 1) * TPB, :], xb)
        if blk == BLK_PER_SEG - 1:
            msr = mlp.tile([1, C], bf16, tag="msr", name=f"msr{s}")
            nc.scalar.copy(msr, ps_seg)   # psum f32 -> bf16 mean row
            nc.gpsimd.dma_start(agin[s:s + 1, :], msr)
    # keep the last few spill writes behind the final load: they fill the
    # DMA gap while the means MLP chain runs
    for nb in range(NBLK - 4, NBLK):
        tile.add_dep_helper(spill_w[nb].ins, last_load[0].ins, sync=True,
                            reason="cluster trailing spill writes in MLP gap")

    # ---- AllGather means
    if collectives:
        nc.gpsimd.collective_compute(
            "AllGather", Alu.bypass, replica_groups=RG,
            ins=[agin.opt()], outs=[agout.opt()],
        )
    else:
        nc.sync.dma_start(agout[:B_LOC, :], agin)
    # meansT via DMA xbar transpose: [64, 512] -> [512, 64] as [128, 4, 64],
    # so partition p / slot j holds feature c = 4p + j (W1 host rows are
    # permuted to match).
    mT_full = mlp.tile([128, 4, B], bf16)
    nc.sync.dma_start_transpose(mT_full, agout)
    mT = [mT_full[:, k, :] for k in range(4)]

    # ---- MLP branch: h1T = W1slice.T @ meansT ; BN per feature ; relu
    def branch(bid, w1_sb, gT, bT):
        haT = []
        for ml in range(FSH // 128):           # 2 local feature tiles
            ph = psB.tile([128, B], f32, tag="ps", name=f"ph{bid}{ml}")
            for k in range(4):
                nc.tensor.matmul(
                    ph, lhsT=w1_sb[k][:, ml * 128:(ml + 1) * 128], rhs=mT[k],
                    start=(k == 0), stop=(k == 3),
                )
            h = mlp.tile([128, B], f32, tag=f"h{bid}{ml}", name=f"h{bid}{ml}")
            s1 = small.tile([128, 1], f32, tag="s1", name=f"s1{bid}{ml}")
            nc.scalar.activation(h, ph, Act.Copy, accum_out=s1)
            sq = small.tile([128, B], f32, tag="sq", name=f"sq{bid}{ml}")
            s2 = small.tile([128, 1], f32, tag="s2", name=f"s2{bid}{ml}")
            nc.scalar.activation(sq, h, Act.Square, bias=zero_col, accum_out=s2)
            mu = small.tile([128, 1], f32, tag="mu", name=f"mu{bid}{ml}")
            nc.scalar.mul(mu, s1, 1.0 / B)
            ex2 = small.tile([128, 1], f32, tag="ex2", name=f"ex2{bid}{ml}")
            nc.scalar.mul(ex2, s2, 1.0 / B)
            # negvar = mu^2 - E[h^2]; std = sqrt(-negvar + eps)
            nv = small.tile([128, 1], f32, tag="nv", name=f"nv{bid}{ml}")
            nc.vector.scalar_tensor_tensor(nv, mu, mu, ex2,
                                           op0=Alu.mult, op1=Alu.subtract)
            std = small.tile([128, 1], f32, tag="std", name=f"std{bid}{ml}")
            nc.scalar.activation(std, nv, Act.Sqrt, bias=eps_col, scale=-1.0)
            istd = small.tile([128, 1], f32, tag="istd", name=f"istd{bid}{ml}")
            nc.vector.reciprocal(istd, std)
            sc = small.tile([128, 1], f32, tag="sc", name=f"sc{bid}{ml}")
            nc.vector.tensor_mul(sc, gT[:, ml:ml + 1], istd)
            t1 = small.tile([128, 1], f32, tag="t1", name=f"t1{bid}{ml}")
            nc.vector.tensor_mul(t1, mu, sc)
            bi = small.tile([128, 1], f32, tag="bi", name=f"bi{bid}{ml}")
            nc.vector.tensor_sub(bi, bT[:, ml:ml + 1], t1)
            ha = mlp.tile([128, B], bf16, tag=f"ha{bid}{ml}", name=f"ha{bid}{ml}")
            nc.scalar.activation(ha, h, Act.Relu, bias=bi, scale=sc)
            haT.append(ha)
        return haT

    haTa = branch("a", w1a_sb, gaT, baT)
    haTb = branch("b", w1b_sb, gbT, bbT)

    # ---- partial second matmuls in [B, C] layout (no output transposes):
    # po[b, c] = sum_mid haT[mid, b] * W2[mid, c]; branch a -> partitions
    # 0..63, branch b -> 64..127 of one PSUM tile.
    po_ps = psB.tile([128, C], f32, tag="ps", name="po_ps")
    for bi_, (w2_sb, haT) in enumerate([(w2a_sb, haTa), (w2b_sb, haTb)]):
        dst = po_ps[bi_ * B:(bi_ + 1) * B, :]
        for ml in range(FSH // 128):
            nc.tensor.matmul(
                dst, lhsT=haT[ml], rhs=w2_sb[ml],
                start=(ml == 0), stop=(ml == FSH // 128 - 1),
            )
    arin_sb = mlp.tile([128, C], bf16)
    nc.scalar.copy(arin_sb, po_ps)
    # dummy sigmoid: forces the sigmoid act-table load into the AllReduce
    # round-trip window (ACT is idle there) instead of the post chain
    nc.scalar.activation(warm, arin_sb[:1, :1], Act.Sigmoid, bias=zero_col[:1])
    arin = dram.tile([128, C], bf16)
    arout = dram.tile([128, C], bf16,
                      addr_space="Shared" if collectives else "Local")
    nc.sync.dma_start(arin, arin_sb)
    if collectives:
        nc.gpsimd.collective_compute(
            "AllReduce", Alu.add, replica_groups=RG,
            ins=[arin.opt()], outs=[arout.opt()],
        )
    else:
        nc.sync.dma_start(arout[:, :], arin)

    # ---- post-AR: bias rows stay raw (relu folds into the bbc broadcast),
    # scale rows get relu+sigmoid (the 0.5*x+0.75 affine folds into sbc).
    post_sb = mlp.tile([128, C], bf16)
    nc.sync.dma_start(post_sb, arout)
    ob = mlp.tile([128, C], bf16, tag="post_ob", name="ob")
    nc.scalar.activation(ob[B:, :], post_sb[B:, :], Act.Relu,
                         bias=zero_col[B:])
    ob2 = mlp.tile([128, C], bf16, tag="post_ob2", name="ob2")
    nc.scalar.activation(ob2[B:, :], ob[B:, :], Act.Sigmoid, bias=zero_col[B:])

    # ---- per-core replicated one-hot selector [128, 8, 128] (bf16):
    # rows 0..63 select the bias row, rows 64..127 the scale row.
    sel_sb = mlp.tile([128, B_LOC, 128], bf16)
    selv = t["sel"].rearrange("(p s) q -> p s q", s=B_LOC)
    nc.sync.dma_start(sel_sb, selv)

    # ---- per-segment broadcast scale/bias tiles, hoisted ahead of the tile
    # loop so segment boundaries never stall the phase-C pipeline (bufs=4
    # gives 4 segments of lookahead).
    # Pool (idle in phase C) does the psum->bf16 copies so they never queue
    # behind ACT's per-tile upconverts; relu/affine fold into them.
    sbcs, bbcs = [], []
    for s in range(B_LOC):
        pbs = psC.tile([128, C], f32, tag="psbc", name=f"pbs{s}")
        nc.tensor.matmul(pbs, lhsT=sel_sb[B:, s, :], rhs=ob2[B:, :],
                         start=True, stop=True)
        sbc = bcp.tile([128, C], bf16, tag="sbc", name=f"sbc{s}")
        nc.gpsimd.tensor_scalar(sbc, pbs, 0.5, 0.75, op0=Alu.mult, op1=Alu.add)
        pbb = psC.tile([128, C], f32, tag="psbc", name=f"pbb{s}")
        nc.tensor.matmul(pbb, lhsT=sel_sb[:B, s, :], rhs=post_sb[:B, :],
                         start=True, stop=True)
        bbc = bcp.tile([128, C], bf16, tag="bbc", name=f"bbc{s}")
        nc.gpsimd.tensor_scalar_max(bbc, pbb, 0.0)
        sbcs.append(sbc)
        bbcs.append(bbc)

    # ---- phase C: out = x_bf16 * scale_bc + bias_bc
    # tile order: residents first (stores flow as soon as the MLP lands),
    # then the spilled tiles (bf16 re-loads prefetch while residents are
    # processed), and one resident tile last so the drain has no re-load.
    # All-bf16 DVE math (2x mode) in place on the x tile, then one ACT copy
    # upconverts to the fp32 store tile: DVE ~2.3us, ACT ~1.9us per tile,
    # both inside the 2.9us per-tile DMA store time.
    order = list(range(NRES - 1)) + list(range(NRES, NBLK)) + [NRES - 1]
    for oi, nb in enumerate(order):
        s, blk = divmod(nb, BLK_PER_SEG)
        sbc_b = sbcs[s][:, None, :].broadcast_to([128, TPB, C])
        bbc_b = bbcs[s][:, None, :].broadcast_to([128, TPB, C])
        if nb < NRES:
            xb = bftiles[nb]
        else:
            j = nb - NRES
            xb = xsp.tile([128, TPB, C], bf16, tag="xsp", name=f"xrl{nb}")
            # Pool-issued: a re-load waiting on its SBUF slot must not block
            # later stores (and vice versa) on the SP sequencer
            nc.gpsimd.dma_start(xb, spill[:, j * TPB:(j + 1) * TPB, :])
        ot = xio.tile([128, TPB, C], f32, tag="xio", name=f"xo{nb}")
        if oi < 2 or oi == NBLK - 1:
            # half-tile granularity on the first/last tiles shortens pipeline
            # fill after the MLP and the final drain
            Hh = TPB // 2
            for hh in range(2):
                sl = slice(hh * Hh, (hh + 1) * Hh)
                nc.vector.tensor_mul(xb[:, sl, :], xb[:, sl, :], sbc_b[:, sl, :])
                nc.vector.tensor_add(xb[:, sl, :], xb[:, sl, :], bbc_b[:, sl, :])
                nc.scalar.copy(ot[:, sl, :], xb[:, sl, :])
                nc.sync.dma_start(
                    ov[:, nb * TPB + hh * Hh:nb * TPB + (hh + 1) * Hh, :],
                    ot[:, sl, :])
        else:
            nc.vector.tensor_mul(xb, xb, sbc_b)      # DVE bf16, in place
            nc.vector.tensor_add(xb, xb, bbc_b)      # DVE bf16, in place
            nc.scalar.copy(ot, xb)                   # ACT bf16 -> f32
            nc.sync.dma_start(ov[:, nb * TPB:(nb + 1) * TPB, :], ot)

    ctx.close()


def _build(num_devices=NCORES, collectives=True):
    key = ("nc", num_devices, collectives)
    if key in _CACHE:
        return _CACHE[key]
    import concourse.bacc as bacc
    import concourse.tile as tile
    from concourse import mybir
    from concourse.masks import make_identity

    f32 = mybir.dt.float32
    bf16 = mybir.dt.bfloat16
    nc = bacc.Bacc("TRN2", target_bir_lowering=False, debug=False,
                   enable_asserts=False, num_devices=num_devices)
    t = {
        "x": nc.dram_tensor("x", [ROWS, C], f32, kind="ExternalInput").ap(),
        "w1a": nc.dram_tensor("w1a", [C, FSH], bf16, kind="ExternalInput").ap(),
        "w2a": nc.dram_tensor("w2a", [FSH, C], bf16, kind="ExternalInput").ap(),
        "w1b": nc.dram_tensor("w1b", [C, FSH], bf16, kind="ExternalInput").ap(),
        "w2b": nc.dram_tensor("w2b", [FSH, C], bf16, kind="ExternalInput").ap(),
        "g1a": nc.dram_tensor("g1a", [FSH], f32, kind="ExternalInput").ap(),
        "b1a": nc.dram_tensor("b1a", [FSH], f32, kind="ExternalInput").ap(),
        "g1b": nc.dram_tensor("g1b", [FSH], f32, kind="ExternalInput").ap(),
        "b1b": nc.dram_tensor("b1b", [FSH], f32, kind="ExternalInput").ap(),
        "sel": nc.dram_tensor("sel", [128 * B_LOC, 128], bf16, kind="ExternalInput").ap(),
        "out": nc.dram_tensor("out", [ROWS, C], f32, kind="ExternalOutput").ap(),
    }
    with tile.TileContext(nc) as tc:
        _emit(nc, tc, tile, mybir, make_identity, t, collectives=collectives)
    nc.compile()
    _CACHE[key] = nc
    return nc


def _make_in_maps(x2, W1a, g1a, b1a, W2a, W1b, g1b, b1b, W2b):
    import ml_dtypes
    bf = ml_dtypes.bfloat16
    # W1 rows permuted to match the dma-transposed means layout: SBUF row
    # j*128 + p holds original input feature c = 4p + j.
    perm = (4 * np.arange(128)[None, :] + np.arange(4)[:, None]).reshape(-1)
    in_maps = []
    for c in range(NCORES):
        f0, f1 = c * FSH, (c + 1) * FSH
        # sel[h*64 + r, s, :] = 1 iff r == c*B_LOC + s  (h=0 bias, h=1 scale)
        sel = np.zeros((2, B, B_LOC, 128), np.float32)
        sel[:, c * B_LOC + np.arange(B_LOC), np.arange(B_LOC), :] = 1.0
        sel = sel.reshape(128 * B_LOC, 128)
        in_maps.append({
            "x": np.ascontiguousarray(x2[c * ROWS:(c + 1) * ROWS]),
            "w1a": np.ascontiguousarray(W1a[perm][:, f0:f1]).astype(bf),
            "w2a": np.ascontiguousarray(W2a[f0:f1, :]).astype(bf),
            "w1b": np.ascontiguousarray(W1b[perm][:, f0:f1]).astype(bf),
            "w2b": np.ascontiguousarray(W2b[f0:f1, :]).astype(bf),
            "g1a": np.ascontiguousarray(g1a[f0:f1]),
            "b1a": np.ascontiguousarray(b1a[f0:f1]),
            "g1b": np.ascontiguousarray(g1b[f0:f1]),
            "b1b": np.ascontiguousarray(b1b[f0:f1]),
            "sel": sel.astype(bf),
        })
    return in_maps


def _numpy_fallback(x2, npoint, W1a, g1a, b1a, W2a, W1b, g1b, b1b, W2b):
    n = x2.shape[0]
    b = npoint.shape[0]
    cum = np.cumsum(npoint)
    seg = np.searchsorted(cum, np.arange(n), side="right")
    counts = npoint.astype(x2.dtype)
    sums = np.zeros((b, x2.shape[1]), x2.dtype)
    np.add.at(sums, seg, x2)
    mean_f = sums / counts[:, None]

    def bn(h, g, bb):
        m = h.mean(0)
        v = h.var(0)
        return (h - m) / np.sqrt(v + EPS) * g + bb

    ha = np.maximum(bn(mean_f @ W1a, g1a, b1a), 0)
    out_mean = np.maximum(ha @ W2a, 0)
    hb = np.maximum(bn(mean_f @ W1b, g1b, b1b), 0)
    zw = np.maximum(hb @ W2b, 0)
    out_w = 1.0 / (1.0 + np.exp(-zw))
    return out_w[seg] * x2 * 0.5 + x2 * 0.75 + out_mean[seg]


def run_on_device(inputs, trace=False, **kwargs):
    """Returns (full_output, BassKernelResults)."""
    from concourse import bass_utils
    x2 = np.asarray(inputs["x2"], np.float32)
    args = {k: np.asarray(inputs[k], np.float32)
            for k in ("W1a", "g1a", "b1a", "W2a", "W1b", "g1b", "b1b", "W2b")}
    nc = _build()
    in_maps = _make_in_maps(x2, args["W1a"], args["g1a"], args["b1a"],
                            args["W2a"], args["W1b"], args["g1b"],
                            args["b1b"], args["W2b"])
    res = bass_utils.run_bass_kernel_spmd(
        nc, in_maps, core_ids=list(range(NCORES)), trace=trace, **kwargs)
    out = np.concatenate([res.results[c]["out"] for c in range(NCORES)], axis=0)
    return out, res


def bench_device(inputs, iters=10, warmup=2, chain=1):
    """Time the sharded NEFF execution with inputs pre-staged on device.

    chain=N runs the kernel N times back-to-back inside one dispatch (each
    call's output feeds the next call's x), so per-call device time can be
    separated from the ~80ms axon dispatch floor via (T(N)-T(1))/(N-1).

    Returns (times_sec_list, output). Mirrors bass2jax.run_bass_via_pjrt's
    multi-core path but without donation so the callable can be re-invoked.
    """
    import time
    import jax
    from jax.experimental.shard_map import shard_map
    from jax.sharding import Mesh, NamedSharding, PartitionSpec
    from concourse import bass2jax, mybir

    nc = _build()
    x2 = np.asarray(inputs["x2"], np.float32)
    args = {k: np.asarray(inputs[k], np.float32)
            for k in ("W1a", "g1a", "b1a", "W2a", "W1b", "g1b", "b1b", "W2b")}
    in_maps = _make_in_maps(x2, args["W1a"], args["g1a"], args["b1a"],
                            args["W2a"], args["W1b"], args["g1b"],
                            args["b1b"], args["W2b"])

    bass2jax.install_neuronx_cc_hook()
    partition_name = (nc.partition_id_tensor.name
                      if nc.partition_id_tensor else None)
    in_names, out_names, out_avals, zero_outs = [], [], [], []
    for alloc in nc.m.functions[0].allocations:
        if not isinstance(alloc, mybir.MemoryLocationSet):
            continue
        name = alloc.memorylocations[0].name
        if alloc.kind == "ExternalInput":
            if name != partition_name:
                in_names.append(name)
        elif alloc.kind == "ExternalOutput":
            shape = tuple(alloc.tensor_shape)
            dtype = mybir.dt.np(alloc.dtype)
            out_names.append(name)
            out_avals.append(jax.core.ShapedArray(shape, dtype))
            zero_outs.append(np.zeros(shape, dtype))
    n_params = len(in_names)
    all_in_names = list(in_names) + list(out_names)
    if partition_name is not None:
        all_in_names.append(partition_name)

    xi = in_names.index("x")

    def _body(*a):
        operands = list(a)
        if partition_name is not None:
            operands.append(bass2jax.partition_id_tensor())
        for _ in range(chain):
            outs = bass2jax._bass_exec_p.bind(
                *operands,
                out_avals=tuple(out_avals),
                in_names=tuple(all_in_names),
                out_names=tuple(out_names),
                lowering_input_output_aliases=(),
                sim_require_finite=True,
                sim_require_nnan=True,
                nc=nc,
            )
            operands[xi] = outs[0]
        return tuple(outs)

    devices = jax.devices()[:NCORES]
    mesh = Mesh(np.asarray(devices), ("core",))
    spec = PartitionSpec("core")
    n_outs = len(out_names)
    fn = jax.jit(
        shard_map(_body, mesh=mesh,
                  in_specs=(spec,) * (n_params + n_outs),
                  out_specs=(spec,) * n_outs, check_rep=False),
        keep_unused=True,
    )
    sharding = NamedSharding(mesh, spec)
    concat_in = [
        jax.device_put(
            np.concatenate([np.asarray(in_maps[c][nm]) for c in range(NCORES)],
                           axis=0), sharding)
        for nm in in_names
    ]
    concat_zero = [
        jax.device_put(np.zeros((NCORES * z.shape[0], *z.shape[1:]), z.dtype),
                       sharding)
        for z in zero_outs
    ]
    for _ in range(warmup):
        r = fn(*concat_in, *concat_zero)
        jax.block_until_ready(r)
    times = []
    for _ in range(iters):
        t0 = time.perf_counter()
        r = fn(*concat_in, *concat_zero)
        jax.block_until_ready(r)
        times.append(time.perf_counter() - t0)
    out = np.asarray(r[0]).reshape(NCORES, ROWS, C).reshape(N, C)
    return times, out


def kernel(**inputs):
    x2 = np.asarray(inputs["x2"], np.float32)
    npoint = np.asarray(inputs["npoint"])
    if (x2.shape != (N, C) or npoint.shape != (B,)
            or not np.all(npoint == SEG)):
        return _numpy_fallback(
            x2, npoint,
            *[np.asarray(inputs[k], np.float32)
              for k in ("W1a", "g1a", "b1a", "W2a", "W1b", "g1b", "b1b", "W2b")],
        ).astype(np.float32)
    out, _ = run_on_device(inputs)
    return out


# revision 40
# speedup vs baseline: 1.0874x; 1.0002x over previous
"""Trainium2 Bass kernel for nn_DCDLayer (ragged_sequence).

Math (see reference):
    mean_f[b]  = mean of x2 rows in segment b                    [B, C]
    ha         = relu(BN(mean_f @ W1a) )  ; out_mean = relu(ha @ W2a)
    hb         = relu(BN(mean_f @ W1b) )  ; out_w    = sigmoid(relu(hb @ W2b))
    out[j]     = x2[j] * (0.5*out_w[seg j] + 0.75) + out_mean[seg j]

Sharding: 8 cores, each owns 8 whole segments (32768 contiguous rows of x2).

The kernel is HBM-bound: it must read x2 (64 MiB/core) and write out
(64 MiB/core); the only tunable traffic is re-materializing x2 for the
combine after the globally-coupled BN stats.  Strategy: during the phase-A
read every x2 tile is downcast to bf16 (rel err ~1e-3, tolerance 2e-2).
NRES tiles stay resident in SBUF; the rest spill to a bf16 DRAM scratch
(half the bytes of an fp32 re-read) and are re-loaded in phase C.

Per-core flow:
  phase A: DMA fp32 tile -> ACT downcast to bf16 -> PE colsum (bf16 matmul
           vs ones into PSUM fp32, accumulated per segment) -> spill-write
           the non-resident bf16 tiles.
  AllGather means [8,512] -> [64,512]  (BatchNorm couples all segments)
  MLP feature-sharded 8-ways (256-wide slice of MID per core, sliced on the
  host into its in_map, weights pre-cast to bf16), BN stats per-feature so
  they stay local; partial second matmuls AllReduce'd ([1024,64], tiny).
  phase C: out = x_bf16 * scale_bc[seg] + bias_bc[seg]
           (DVE mul + Pool add per tile), resident tiles first, then the
           spilled tiles re-loaded from the bf16 scratch.
"""

import sys
import numpy as np

for _p in ("/opt/trn_rl_repo",):
    if _p not in sys.path:
        sys.path.insert(0, _p)

B = 64            # segments
SEG = 4096        # rows per segment
N = B * SEG
C = 512
MID = 2048
EPS = 1e-5

NCORES = 8
B_LOC = B // NCORES          # 8 segments per core
ROWS = N // NCORES           # 32768 rows per core
FSH = MID // NCORES          # 256 features of MID per core
TPB = 4                      # 128-row tiles per DMA block (1 MiB fp32 blocks)
BLK_PER_SEG = SEG // (128 * TPB)   # 8 blocks per segment
NBLK = ROWS // (128 * TPB)   # 64 blocks per core
NRES = 35                    # bf16 tiles kept resident in SBUF
NSPILL = NBLK - NRES         # bf16 tiles spilled to DRAM scratch

_CACHE = {}


def _emit(nc, tc, tile, mybir, make_identity, t, collectives=True):
    f32 = mybir.dt.float32
    f32r = mybir.dt.float32r
    bf16 = mybir.dt.bfloat16
    Alu = mybir.AluOpType
    Act = mybir.ActivationFunctionType
    X = mybir.AxisListType.X
    RG = [list(range(NCORES))]

    from contextlib import ExitStack
    ctx = ExitStack()
    consts = ctx.enter_context(tc.tile_pool(name="consts", bufs=1))
    wpool = ctx.enter_context(tc.tile_pool(name="wpool", bufs=1))
    mlp = ctx.enter_context(tc.tile_pool(name="mlp", bufs=1))
    small = ctx.enter_context(tc.tile_pool(name="small", bufs=2))
    xio = ctx.enter_context(tc.tile_pool(name="xio", bufs=3))
    xsp = ctx.enter_context(tc.tile_pool(name="xsp", bufs=4))
    resp = ctx.enter_context(tc.tile_pool(name="resp", bufs=NRES))
    bcp = ctx.enter_context(tc.tile_pool(name="bcp", bufs=4))
    psA = ctx.enter_context(tc.tile_pool(name="psA", bufs=1, space="PSUM"))
    psB = ctx.enter_context(tc.tile_pool(name="psB", bufs=3, space="PSUM"))
    psC = ctx.enter_context(tc.tile_pool(name="psC", bufs=4, space="PSUM"))
    dram = ctx.enter_context(tc.tile_pool(name="dram", bufs=1, space="DRAM"))

    # ---- constants
    ones_bf = consts.tile([128, 1], bf16)   # 1/SEG folds the mean into colsum
    nc.gpsimd.memset(ones_bf, 1.0 / SEG)    # 2^-12, exact in bf16
    eps_col = consts.tile([128, 1], f32)
    nc.gpsimd.memset(eps_col, EPS)
    zero_col = consts.tile([128, 1], f32)
    nc.gpsimd.memset(zero_col, 0.0)
    # preload act tables: Sigmoid first, then Sqrt, so the sqrt table (which
    # also serves Copy/Square used through phase A) is resident when the BN
    # std is computed -- no table load on the means->scale critical chain.
    warm = consts.tile([1, 1], f32)
    nc.scalar.activation(warm, zero_col[:1, :], Act.Sigmoid, bias=zero_col[:1, :])
    nc.scalar.activation(warm, zero_col[:1, :], Act.Sqrt, bias=eps_col[:1, :])

    # ---- weights (per-core feature slices, bf16 from host) -> SBUF
    def load_w(name, ap, p_tiles, fdim):
        out = []
        for k in range(p_tiles):
            w = wpool.tile([128, fdim], bf16, tag=f"{name}{k}", name=f"{name}{k}")
            nc.sync.dma_start(w, ap[k * 128:(k + 1) * 128, :])
            out.append(w)
        return out

    w1a_sb = load_w("w1a", t["w1a"], 4, FSH)   # [512,256] -> 4x[128,256]
    w1b_sb = load_w("w1b", t["w1b"], 4, FSH)
    w2a_sb = load_w("w2a", t["w2a"], 2, C)     # [256,512] -> 2x[128,512]
    w2b_sb = load_w("w2b", t["w2b"], 2, C)

    def load_gb(name, vec):   # dram [FSH] -> SBUF [128, FSH//128] (feature on partition)
        o = mlp.tile([128, FSH // 128], f32, tag=f"{name}T", name=f"{name}T")
        nc.sync.dma_start(o, vec.rearrange("(a b) -> b a", a=FSH // 128))
        return o

    gaT = load_gb("ga", t["g1a"])
    baT = load_gb("ba", t["b1a"])
    gbT = load_gb("gb", t["g1b"])
    bbT = load_gb("bb", t["b1b"])

    xv = t["x"].rearrange("(n p) c -> p n c", p=128)    # [128, 256, 512]
    ov = t["out"].rearrange("(n p) c -> p n c", p=128)

    # bf16 DRAM scratch for the NSPILL spilled tiles
    spill = dram.tile([128, NSPILL * TPB, C], bf16)

    # ---- phase A: stream fp32, downcast to bf16, PE colsum per segment
    # tiles 0..NRES-1 resident; tiles NRES..NBLK-1 spill (their writes trail
    # the last phase-A load, filling the DMA gap while the MLP runs).
    bftiles = {}
    agin = dram.tile([B_LOC, C], bf16)
    agout = dram.tile([B, C], bf16,
                      addr_space="Shared" if collectives else "Local")
    ps_seg = None
    spill_w = {}
    last_load = [None]
    for nb in range(NBLK):
        s, blk = divmod(nb, BLK_PER_SEG)
        xt = xio.tile([128, TPB, C], f32, tag="xio", name=f"xa{nb}")
        last_load[0] = nc.sync.dma_start(xt, xv[:, nb * TPB:(nb + 1) * TPB, :])
        if nb < NRES:
            xb = resp.tile([128, TPB, C], bf16, tag="res", name=f"xres{nb}")
        else:
            xb = xsp.tile([128, TPB, C], bf16, tag="xsp", name=f"xsp{nb}")
        nc.scalar.copy(xb, xt)          # ACT fp32 -> bf16
        bftiles[nb] = xb
        if blk == 0:
            ps_seg = psA.tile([1, C], f32, tag="psA", name=f"psA{s}")
        # colsum off the bf16 tile (1 cycle/row on PE, fp32 PSUM accumulate)
        for k in range(TPB):
            nc.tensor.matmul(ps_seg, lhsT=ones_bf, rhs=xb[:, k, :],
                             start=(blk == 0 and k == 0),
                             stop=(blk == BLK_PER_SEG - 1 and k == TPB - 1))
        if nb >= NRES:
            j = nb - NRES
            # Pool-issued (SWDGE): a spill write waiting on its downcast must
            # not block later phase-A loads on the SP sequencer
            spill_w[nb] = nc.gpsimd.dma_start(
                spill[:, j * TPB:(j + 1) * TPB, :], xb)
        if blk == BLK_PER_SEG - 1:
            msr = mlp.tile([1, C], bf16, tag="msr", name=f"msr{s}")
            nc.scalar.copy(msr, ps_seg)   # psum f32 -> bf16 mean row
            nc.gpsimd.dma_start(agin[s:s + 1, :], msr)
    # keep the last few spill writes behind the final load: they fill the
    # DMA gap while the means MLP chain runs
    for nb in range(NBLK - 4, NBLK):
        tile.add_dep_helper(spill_w[nb].ins, last_load[0].ins, sync=True,
                            reason="cluster trailing spill writes in MLP gap")

    # ---- AllGather means
    if collectives:
        nc.gpsimd.collective_compute(
            "AllGather", Alu.bypass, replica_groups=RG,
            ins=[agin.opt()], outs=[agout.opt()],
        )
    else:
        nc.sync.dma_start(agout[:B_LOC, :], agin)
    # meansT via DMA xbar transpose: [64, 512] -> [512, 64] as [128, 4, 64],
    # so partition p / slot j holds feature c = 4p + j (W1 host rows are
    # permuted to match).
    mT_full = mlp.tile([128, 4, B], bf16)
    nc.sync.dma_start_transpose(mT_full, agout)
    mT = [mT_full[:, k, :] for k in range(4)]

    # ---- MLP branch: h1T = W1slice.T @ meansT ; BN per feature ; relu
    def branch(bid, w1_sb, gT, bT):
        haT = []
        for ml in range(FSH // 128):           # 2 local feature tiles
            ph = psB.tile([128, B], f32, tag="ps", name=f"ph{bid}{ml}")
            for k in range(4):
                nc.tensor.matmul(
                    ph, lhsT=w1_sb[k][:, ml * 128:(ml + 1) * 128], rhs=mT[k],
                    start=(k == 0), stop=(k == 3),
                )
            h = mlp.tile([128, B], f32, tag=f"h{bid}{ml}", name=f"h{bid}{ml}")
            s1 = small.tile([128, 1], f32, tag="s1", name=f"s1{bid}{ml}")
            nc.scalar.activation(h, ph, Act.Copy, accum_out=s1)
            sq = small.tile([128, B], f32, tag="sq", name=f"sq{bid}{ml}")
            s2 = small.tile([128, 1], f32, tag="s2", name=f"s2{bid}{ml}")
            nc.scalar.activation(sq, h, Act.Square, bias=zero_col, accum_out=s2)
            mu = small.tile([128, 1], f32, tag="mu", name=f"mu{bid}{ml}")
            nc.scalar.mul(mu, s1, 1.0 / B)
            ex2 = small.tile([128, 1], f32, tag="ex2", name=f"ex2{bid}{ml}")
            nc.scalar.mul(ex2, s2, 1.0 / B)
            # negvar = mu^2 - E[h^2]; std = sqrt(-negvar + eps)
            nv = small.tile([128, 1], f32, tag="nv", name=f"nv{bid}{ml}")
            nc.vector.scalar_tensor_tensor(nv, mu, mu, ex2,
                                           op0=Alu.mult, op1=Alu.subtract)
            std = small.tile([128, 1], f32, tag="std", name=f"std{bid}{ml}")
            nc.scalar.activation(std, nv, Act.Sqrt, bias=eps_col, scale=-1.0)
            istd = small.tile([128, 1], f32, tag="istd", name=f"istd{bid}{ml}")
            nc.vector.reciprocal(istd, std)
            sc = small.tile([128, 1], f32, tag="sc", name=f"sc{bid}{ml}")
            nc.vector.tensor_mul(sc, gT[:, ml:ml + 1], istd)
            t1 = small.tile([128, 1], f32, tag="t1", name=f"t1{bid}{ml}")
            nc.vector.tensor_mul(t1, mu, sc)
            bi = small.tile([128, 1], f32, tag="bi", name=f"bi{bid}{ml}")
            nc.vector.tensor_sub(bi, bT[:, ml:ml + 1], t1)
            ha = mlp.tile([128, B], bf16, tag=f"ha{bid}{ml}", name=f"ha{bid}{ml}")
            nc.scalar.activation(ha, h, Act.Relu, bias=bi, scale=sc)
            haT.append(ha)
        return haT

    haTa = branch("a", w1a_sb, gaT, baT)
    haTb = branch("b", w1b_sb, gbT, bbT)

    # ---- partial second matmuls in [B, C] layout (no output transposes):
    # po[b, c] = sum_mid haT[mid, b] * W2[mid, c]; branch a -> partitions
    # 0..63, branch b -> 64..127 of one PSUM tile.
    po_ps = psB.tile([128, C], f32, tag="ps", name="po_ps")
    for bi_, (w2_sb, haT) in enumerate([(w2a_sb, haTa), (w2b_sb, haTb)]):
        dst = po_ps[bi_ * B:(bi_ + 1) * B, :]
        for ml in range(FSH // 128):
            nc.tensor.matmul(
                dst, lhsT=haT[ml], rhs=w2_sb[ml],
                start=(ml == 0), stop=(ml == FSH // 128 - 1),
            )
    arin_sb = mlp.tile([128, C], bf16)
    nc.scalar.copy(arin_sb, po_ps)
    # dummy sigmoid: forces the sigmoid act-table load into the AllReduce
    # round-trip window (ACT is idle there) instead of the post chain
    nc.scalar.activation(warm, arin_sb[:1, :1], Act.Sigmoid, bias=zero_col[:1])
    arin = dram.tile([128, C], bf16)
    arout = dram.tile([128, C], bf16,
                      addr_space="Shared" if collectives else "Local")
    nc.sync.dma_start(arin, arin_sb)
    if collectives:
        nc.gpsimd.collective_compute(
            "AllReduce", Alu.add, replica_groups=RG,
            ins=[arin.opt()], outs=[arout.opt()],
        )
    else:
        nc.sync.dma_start(arout[:, :], arin)

    # ---- post-AR: bias rows stay raw (relu folds into the bbc broadcast),
    # scale rows get relu+sigmoid (the 0.5*x+0.75 affine folds into sbc).
    post_sb = mlp.tile([128, C], bf16)
    nc.sync.dma_start(post_sb, arout)
    ob = mlp.tile([128, C], bf16, tag="post_ob", name="ob")
    nc.scalar.activation(ob[B:, :], post_sb[B:, :], Act.Relu,
                         bias=zero_col[B:])
    ob2 = mlp.tile([128, C], bf16, tag="post_ob2", name="ob2")
    nc.scalar.activation(ob2[B:, :], ob[B:, :], Act.Sigmoid, bias=zero_col[B:])

    # ---- per-core replicated one-hot selector [128, 8, 128] (bf16):
    # rows 0..63 select the bias row, rows 64..127 the scale row.
    sel_sb = mlp.tile([128, B_LOC, 128], bf16)
    selv = t["sel"].rearrange("(p s) q -> p s q", s=B_LOC)
    nc.sync.dma_start(sel_sb, selv)

    # ---- per-segment broadcast scale/bias tiles, hoisted ahead of the tile
    # loop so segment boundaries never stall the phase-C pipeline (bufs=4
    # gives 4 segments of lookahead).
    # Pool (idle in phase C) does the psum->bf16 copies so they never queue
    # behind ACT's per-tile upconverts; relu/affine fold into them.
    sbcs, bbcs = [], []
    for s in range(B_LOC):
        pbs = psC.tile([128, C], f32, tag="psbc", name=f"pbs{s}")
        nc.tensor.matmul(pbs, lhsT=sel_sb[B:, s, :], rhs=ob2[B:, :],
                         start=True, stop=True)
        sbc = bcp.tile([128, C], bf16, tag="sbc", name=f"sbc{s}")
        nc.gpsimd.tensor_scalar(sbc, pbs, 0.5, 0.75, op0=Alu.mult, op1=Alu.add)
        pbb = psC.tile([128, C], f32, tag="psbc", name=f"pbb{s}")
        nc.tensor.matmul(pbb, lhsT=sel_sb[:B, s, :], rhs=post_sb[:B, :],
                         start=True, stop=True)
        bbc = bcp.tile([128, C], bf16, tag="bbc", name=f"bbc{s}")
        nc.gpsimd.tensor_scalar_max(bbc, pbb, 0.0)
        sbcs.append(sbc)
        bbcs.append(bbc)

    # ---- phase C: out = x_bf16 * scale_bc + bias_bc
    # tile order: residents first (stores flow as soon as the MLP lands),
    # then the spilled tiles (bf16 re-loads prefetch while residents are
    # processed), and one resident tile last so the drain has no re-load.
    # All-bf16 DVE math (2x mode) in place on the x tile, then one ACT copy
    # upconverts to the fp32 store tile: DVE ~2.3us, ACT ~1.9us per tile,
    # both inside the 2.9us per-tile DMA store time.
    order = list(range(NRES - 1)) + list(range(NRES, NBLK)) + [NRES - 1]
    for oi, nb in enumerate(order):
        s, blk = divmod(nb, BLK_PER_SEG)
        sbc_b = sbcs[s][:, None, :].broadcast_to([128, TPB, C])
        bbc_b = bbcs[s][:, None, :].broadcast_to([128, TPB, C])
        if nb < NRES:
            xb = bftiles[nb]
        else:
            j = nb - NRES
            xb = xsp.tile([128, TPB, C], bf16, tag="xsp", name=f"xrl{nb}")
            # Pool-issued: a re-load waiting on its SBUF slot must not block
            # later stores (and vice versa) on the SP sequencer
            nc.gpsimd.dma_start(xb, spill[:, j * TPB:(j + 1) * TPB, :])
        ot = xio.tile([128, TPB, C], f32, tag="xio", name=f"xo{nb}")
        if oi < 2 or oi == NBLK - 1:
            # half-tile granularity on the first/last tiles shortens pipeline
            # fill after the MLP and the final drain
            Hh = TPB // 2
            for hh in range(2):
                sl = slice(hh * Hh, (hh + 1) * Hh)
                nc.vector.tensor_mul(xb[:, sl, :], xb[:, sl, :], sbc_b[:, sl, :])
                nc.vector.tensor_add(xb[:, sl, :], xb[:, sl, :], bbc_b[:, sl, :])
                nc.scalar.copy(ot[:, sl, :], xb[:, sl, :])
                nc.sync.dma_start(
                    ov[:, nb * TPB + hh * Hh:nb * TPB + (hh + 1) * Hh, :],
                    ot[:, sl, :])
        else:
            nc.vector.tensor_mul(xb, xb, sbc_b)      # DVE bf16, in place
            nc.vector.tensor_add(xb, xb, bbc_b)      # DVE bf16, in place
            nc.scalar.copy(ot, xb)                   # ACT bf16 -> f32
            nc.sync.dma_start(ov[:, nb * TPB:(nb + 1) * TPB, :], ot)

    ctx.close()


def _build(num_devices=NCORES, collectives=True):
    key = ("nc", num_devices, collectives)
    if key in _CACHE:
        return _CACHE[key]
    import concourse.bacc as bacc
    import concourse.tile as tile
    from concourse import mybir
    from concourse.masks import make_identity

    f32 = mybir.dt.float32
    bf16 = mybir.dt.bfloat16
    nc = bacc.Bacc("TRN2", target_bir_lowering=False, debug=False,
                   enable_asserts=False, num_devices=num_devices)
    t = {
        "x": nc.dram_tensor("x", [ROWS, C], f32, kind="ExternalInput").ap(),
        "w1a": nc.dram_tensor("w1a", [C, FSH], bf16, kind="ExternalInput").ap(),
        "w2a": nc.dram_tensor("w2a", [FSH, C], bf16, kind="ExternalInput").ap(),
        "w1b": nc.dram_tensor("w1b", [C, FSH], bf16, kind="ExternalInput").ap(),
        "w2b": nc.dram_tensor("w2b", [FSH, C], bf16, kind="ExternalInput").ap(),
        "g1a": nc.dram_tensor("g1a", [FSH], f32, kind="ExternalInput").ap(),
        "b1a": nc.dram_tensor("b1a", [FSH], f32, kind="ExternalInput").ap(),
        "g1b": nc.dram_tensor("g1b", [FSH], f32, kind="ExternalInput").ap(),
        "b1b": nc.dram_tensor("b1b", [FSH], f32, kind="ExternalInput").ap(),
        "sel": nc.dram_tensor("sel", [128 * B_LOC, 128], bf16, kind="ExternalInput").ap(),
        "out": nc.dram_tensor("out", [ROWS, C], f32, kind="ExternalOutput").ap(),
    }
    with tile.TileContext(nc) as tc:
        _emit(nc, tc, tile, mybir, make_identity, t, collectives=collectives)
    nc.compile()
    _CACHE[key] = nc
    return nc


def _make_in_maps(x2, W1a, g1a, b1a, W2a, W1b, g1b, b1b, W2b):
    import ml_dtypes
    bf = ml_dtypes.bfloat16
    # W1 rows permuted to match the dma-transposed means layout: SBUF row
    # j*128 + p holds original input feature c = 4p + j.
    perm = (4 * np.arange(128)[None, :] + np.arange(4)[:, None]).reshape(-1)
    in_maps = []
    for c in range(NCORES):
        f0, f1 = c * FSH, (c + 1) * FSH
        # sel[h*64 + r, s, :] = 1 iff r == c*B_LOC + s  (h=0 bias, h=1 scale)
        sel = np.zeros((2, B, B_LOC, 128), np.float32)
        sel[:, c * B_LOC + np.arange(B_LOC), np.arange(B_LOC), :] = 1.0
        sel = sel.reshape(128 * B_LOC, 128)
        in_maps.append({
            "x": np.ascontiguousarray(x2[c * ROWS:(c + 1) * ROWS]),
            "w1a": np.ascontiguousarray(W1a[perm][:, f0:f1]).astype(bf),
            "w2a": np.ascontiguousarray(W2a[f0:f1, :]).astype(bf),
            "w1b": np.ascontiguousarray(W1b[perm][:, f0:f1]).astype(bf),
            "w2b": np.ascontiguousarray(W2b[f0:f1, :]).astype(bf),
            "g1a": np.ascontiguousarray(g1a[f0:f1]),
            "b1a": np.ascontiguousarray(b1a[f0:f1]),
            "g1b": np.ascontiguousarray(g1b[f0:f1]),
            "b1b": np.ascontiguousarray(b1b[f0:f1]),
            "sel": sel.astype(bf),
        })
    return in_maps


def _numpy_fallback(x2, npoint, W1a, g1a, b1a, W2a, W1b, g1b, b1b, W2b):
    n = x2.shape[0]
    b = npoint.shape[0]
    cum = np.cumsum(npoint)
    seg = np.searchsorted(cum, np.arange(n), side="right")
    counts = npoint.astype(x2.dtype)
    sums = np.zeros((b, x2.shape[1]), x2.dtype)
    np.add.at(sums, seg, x2)
    mean_f = sums / counts[:, None]

    def bn(h, g, bb):
        m = h.mean(0)
        v = h.var(0)
        return (h - m) / np.sqrt(v + EPS) * g + bb

    ha = np.maximum(bn(mean_f @ W1a, g1a, b1a), 0)
    out_mean = np.maximum(ha @ W2a, 0)
    hb = np.maximum(bn(mean_f @ W1b, g1b, b1b), 0)
    zw = np.maximum(hb @ W2b, 0)
    out_w = 1.0 / (1.0 + np.exp(-zw))
    return out_w[seg] * x2 * 0.5 + x2 * 0.75 + out_mean[seg]


def run_on_device(inputs, trace=False, **kwargs):
    """Returns (full_output, BassKernelResults)."""
    from concourse import bass_utils
    x2 = np.asarray(inputs["x2"], np.float32)
    args = {k: np.asarray(inputs[k], np.float32)
            for k in ("W1a", "g1a", "b1a", "W2a", "W1b", "g1b", "b1b", "W2b")}
    nc = _build()
    in_maps = _make_in_maps(x2, args["W1a"], args["g1a"], args["b1a"],
                            args["W2a"], args["W1b"], args["g1b"],
                            args["b1b"], args["W2b"])
    res = bass_utils.run_bass_kernel_spmd(
        nc, in_maps, core_ids=list(range(NCORES)), trace=trace, **kwargs)
    out = np.concatenate([res.results[c]["out"] for c in range(NCORES)], axis=0)
    return out, res


def bench_device(inputs, iters=10, warmup=2, chain=1):
    """Time the sharded NEFF execution with inputs pre-staged on device.

    chain=N runs the kernel N times back-to-back inside one dispatch (each
    call's output feeds the next call's x), so per-call device time can be
    separated from the ~80ms axon dispatch floor via (T(N)-T(1))/(N-1).

    Returns (times_sec_list, output). Mirrors bass2jax.run_bass_via_pjrt's
    multi-core path but without donation so the callable can be re-invoked.
    """
    import time
    import jax
    from jax.experimental.shard_map import shard_map
    from jax.sharding import Mesh, NamedSharding, PartitionSpec
    from concourse import bass2jax, mybir

    nc = _build()
    x2 = np.asarray(inputs["x2"], np.float32)
    args = {k: np.asarray(inputs[k], np.float32)
            for k in ("W1a", "g1a", "b1a", "W2a", "W1b", "g1b", "b1b", "W2b")}
    in_maps = _make_in_maps(x2, args["W1a"], args["g1a"], args["b1a"],
                            args["W2a"], args["W1b"], args["g1b"],
                            args["b1b"], args["W2b"])

    bass2jax.install_neuronx_cc_hook()
    partition_name = (nc.partition_id_tensor.name
                      if nc.partition_id_tensor else None)
    in_names, out_names, out_avals, zero_outs = [], [], [], []
    for alloc in nc.m.functions[0].allocations:
        if not isinstance(alloc, mybir.MemoryLocationSet):
            continue
        name = alloc.memorylocations[0].name
        if alloc.kind == "ExternalInput":
            if name != partition_name:
                in_names.append(name)
        elif alloc.kind == "ExternalOutput":
            shape = tuple(alloc.tensor_shape)
            dtype = mybir.dt.np(alloc.dtype)
            out_names.append(name)
            out_avals.append(jax.core.ShapedArray(shape, dtype))
            zero_outs.append(np.zeros(shape, dtype))
    n_params = len(in_names)
    all_in_names = list(in_names) + list(out_names)
    if partition_name is not None:
        all_in_names.append(partition_name)

    xi = in_names.index("x")

    def _body(*a):
        operands = list(a)
        if partition_name is not None:
            operands.append(bass2jax.partition_id_tensor())
        for _ in range(chain):
            outs = bass2jax._bass_exec_p.bind(
                *operands,
                out_avals=tuple(out_avals),
                in_names=tuple(all_in_names),
                out_names=tuple(out_names),
                lowering_input_output_aliases=(),
                sim_require_finite=True,
                sim_require_nnan=True,
                nc=nc,
            )
            operands[xi] = outs[0]
        return tuple(outs)

    devices = jax.devices()[:NCORES]
    mesh = Mesh(np.asarray(devices), ("core",))
    spec = PartitionSpec("core")
    n_outs = len(out_names)
    fn = jax.jit(
        shard_map(_body, mesh=mesh,
                  in_specs=(spec,) * (n_params + n_outs),
                  out_specs=(spec,) * n_outs, check_rep=False),
        keep_unused=True,
    )
    sharding = NamedSharding(mesh, spec)
    concat_in = [
        jax.device_put(
            np.concatenate([np.asarray(in_maps[c][nm]) for c in range(NCORES)],
                           axis=0), sharding)
        for nm in in_names
    ]
    concat_zero = [
        jax.device_put(np.zeros((NCORES * z.shape[0], *z.shape[1:]), z.dtype),
                       sharding)
        for z in zero_outs
    ]
    for _ in range(warmup):
        r = fn(*concat_in, *concat_zero)
        jax.block_until_ready(r)
    times = []
    for _ in range(iters):
        t0 = time.perf_counter()
        r = fn(*concat_in, *concat_zero)
        jax.block_until_ready(r)
        times.append(time.perf_counter() - t0)
    out = np.asarray(r[0]).reshape(NCORES, ROWS, C).reshape(N, C)
    return times, out


def kernel(**inputs):
    x2 = np.asarray(inputs["x2"], np.float32)
    npoint = np.asarray(inputs["npoint"])
    if (x2.shape != (N, C) or npoint.shape != (B,)
            or not np.all(npoint == SEG)):
        return _numpy_fallback(
            x2, npoint,
            *[np.asarray(inputs[k], np.float32)
              for k in ("W1a", "g1a", "b1a", "W2a", "W1b", "g1b", "b1b", "W2b")],
        ).astype(np.float32)
    out, _ = run_on_device(inputs)
    return out
